# revision 13
# baseline (speedup 1.0000x reference)
"""Masked attention-weight kernel (dense_transformer) for 8 TRN2 NeuronCores.

Computes, for inputs query/key [32,1024,512] f32, masks [32,1024] i32:
    q = relu(query @ Wq + bq); k = relu(key @ Wk + bk)
    w = softmax((q @ k^T)/sqrt(512) + key_mask_additive) * query_mask
Output: [32, 1024, 1024] f32.

Strategy: data-parallel over batch (4 batches/core, no collectives) PLUS
host-side mask compaction.  Masked key columns have weight exactly 0 in the
reference (exp(-1e9) underflows) and masked query rows are zeroed, so the
host gathers only the valid ~512 query rows / key columns per batch, pads
them to a fixed NQP/NKP (multiple of 128, 640 for this data), and the device
runs dense attention on the compacted [NQP, NKP] problem -- ~2.2x fewer
matmul cycles than the full [1024,1024].  The host scatters the compact
output back into a zero-filled full-size array.

Padded key columns are all-zero inputs, so (with zero bias -- true for this
problem) their projected features are 0, their logits are 0, and each
contributes exp(0)=1 to the softmax row-sum; the device subtracts the
host-provided pad count from the row-sum before taking the reciprocal.
If the key bias were nonzero the host instead ships an additive -1e4
column mask applied to the projected k (use_mask variant).

Per-core pipeline, per batch (all matmuls bf16 with f32 PSUM):
  1. kTm[e,j] = relu(Wk.T @ keyT + bk): PE matmuls in (512,128) psum-bank
     chunks -> relu+bias epilogue alternating ACT/DVE.
  2. qT[e,i] likewise.  Batch 0 runs both projections dt-major across 8
     open psum chains so the PE consumes (w_dt, x_dt) DMA pairs in arrival
     order at cold start.
  3. Per 128-row block: S = qT.T @ kTm (PE), ACT exp with fused row-sum,
     DVE pad-correction + reciprocal, DVE scale, DMA out (stores alternate
     between the gpsimd and sync queues).
"""

import sys

sys.path.insert(0, "/opt/trn_rl_repo")

import numpy as np
import ml_dtypes
from contextlib import ExitStack

import concourse.tile as tile
from concourse import bacc, mybir
from concourse.bass_utils import run_bass_kernel_spmd

P = 128
B, LQ, LK, D = 32, 1024, 1024, 512
NCORES = 8
BL = B // NCORES          # batches per core
NDT = D // P              # contraction tiles for projections
NET = D // P              # output-feature tiles (= S contraction tiles)
SCALE = float(1.0 / np.sqrt(D))
MASKC = -1.0e4

F32 = mybir.dt.float32
BF16 = mybir.dt.bfloat16
FP8 = mybir.dt.float8e4
AF = mybir.ActivationFunctionType

_CACHE = {}


def _chunks(width):
    """Split a free width into psum-bank-aligned chunks (<=512 each)."""
    out, c0 = [], 0
    while c0 < width:
        cw = min(512, width - c0)
        out.append((c0, cw))
        c0 += cw
    return out


def _body(tc, qT, kT, Wq, Wk, bq, bk, padc, maskc, out, NQP, NKP):
    nc = tc.nc
    # fp8 DoubleRow S-matmul measured L2 err 1.9e-2 vs the 2e-2 gate --
    # only ~3us faster than bf16 (S phase is ACT-bound), so keep bf16.
    s_fp8 = False
    NQB = (NQP + P - 1) // P  # S blocks per batch (last may be 64 rows)
    rows_of = lambda ib: min(P, NQP - ib * P)
    SPAD = ((NKP + 511) // 512) * 512   # psum tile width (bank aligned)
    kchunks = _chunks(NKP)
    qchunks = _chunks(NQP)
    use_mask = maskc is not None
    with ExitStack() as ctx:
        consts = ctx.enter_context(tc.tile_pool(name="consts", bufs=1))
        wpool = ctx.enter_context(tc.tile_pool(name="w", bufs=1))
        inpool = ctx.enter_context(tc.tile_pool(name="inp", bufs=2))
        actpool = ctx.enter_context(tc.tile_pool(name="act", bufs=2))
        mpool = ctx.enter_context(tc.tile_pool(name="mask", bufs=2))
        epool = ctx.enter_context(tc.tile_pool(name="exp", bufs=3))
        opool = ctx.enter_context(tc.tile_pool(name="pout", bufs=3))
        stpool = ctx.enter_context(tc.tile_pool(name="stat", bufs=6))
        ppsum = ctx.enter_context(tc.tile_pool(name="ppsum", bufs=3, space="PSUM"))
        spsum = ctx.enter_context(tc.tile_pool(name="spsum", bufs=2, space="PSUM"))
        cpsum = ctx.enter_context(tc.tile_pool(name="cpsum", bufs=1, space="PSUM"))

        # Weights on the scalar DMA queue, inputs on sync, small tensors on
        # gpsimd -- three queues pull concurrently at cold start.
        wk_sb = [wpool.tile([P, D], BF16, tag=f"wk{dt_}", name=f"wk{dt_}")
                 for dt_ in range(NDT)]
        wq_sb = [wpool.tile([P, D], BF16, tag=f"wq{dt_}", name=f"wq{dt_}")
                 for dt_ in range(NDT)]
        for dt_ in range(NDT):
            nc.scalar.dma_start(
                out=wk_sb[dt_][:], in_=Wk[dt_ * P:(dt_ + 1) * P, :])
        for dt_ in range(NDT):
            nc.scalar.dma_start(
                out=wq_sb[dt_][:], in_=Wq[dt_ * P:(dt_ + 1) * P, :])

        bk_sb = consts.tile([P, NET], F32)
        nc.gpsimd.dma_start(out=bk_sb[:], in_=bk[:])
        bq_sb = consts.tile([P, NET], F32)
        nc.gpsimd.dma_start(out=bq_sb[:], in_=bq[:])

        # PE warmup: 8 dummy matmuls (~3.4us of cold PE busy, exactly one
        # HAM activity window) on scratch tiles while the first input DMAs
        # are in flight, so the clock-gate reaches K=8/8 just before real
        # matmuls start. Results are never read.
        warm_in = consts.tile([P, 512], BF16, name="warm_in")
        nc.vector.memset(warm_in[:], 0.0)
        warm_ps = ppsum.tile([P, 512], F32, tag="proj", name="warm_ps")
        for _ in range(4):
            nc.tensor.matmul(
                warm_ps[:], lhsT=warm_in[:, 0:P], rhs=warm_in[:],
                start=True, stop=True,
            )

        def load_inputs(b):
            xk, xq = [], []
            for dt_ in range(NDT):
                t = inpool.tile([P, NKP], BF16, tag=f"xk{dt_}")
                if b == 0 and dt_ == 0:
                    # split tile 0 so the first matmul's dep lands sooner;
                    # whole-tile for the rest (each descriptor costs ~600ns
                    # of queue issue time, which is the cold-start gate)
                    for (c0, cw) in kchunks:
                        nc.sync.dma_start(
                            out=t[:, c0:c0 + cw],
                            in_=kT[b, dt_ * P:(dt_ + 1) * P, c0:c0 + cw])
                else:
                    nc.sync.dma_start(
                        out=t[:], in_=kT[b, dt_ * P:(dt_ + 1) * P, :])
                xk.append(t)
            for dt_ in range(NDT):
                t = inpool.tile([P, NQP], BF16, tag=f"xq{dt_}")
                if b == 0:
                    # gpsimd queue: nearly idle at cold start, so xq doesn't
                    # serialize behind the xk loads on sync
                    if dt_ == 0:
                        for (c0, cw) in qchunks:
                            nc.gpsimd.dma_start(
                                out=t[:, c0:c0 + cw],
                                in_=qT[b, dt_ * P:(dt_ + 1) * P, c0:c0 + cw])
                    else:
                        nc.gpsimd.dma_start(
                            out=t[:], in_=qT[b, dt_ * P:(dt_ + 1) * P, :])
                else:
                    nc.sync.dma_start(
                        out=t[:], in_=qT[b, dt_ * P:(dt_ + 1) * P, :])
                xq.append(t)
            pad_sb = mpool.tile([P, 1], F32, tag="padc")
            nc.gpsimd.dma_start(out=pad_sb[:], in_=padc[b])
            mask_sb = None
            if use_mask:
                mask_sb = mpool.tile([P, NKP], BF16, tag="maskc")
                nc.gpsimd.dma_start(out=mask_sb[:], in_=maskc[b])
            return xk, xq, pad_sb, mask_sb

        def _eslice(out_tiles, et, c0, cw):
            # fp8 mode packs et pairs into [P, 2, N] DoubleRow operand tiles
            if s_fp8:
                return out_tiles[et // 2][:, et % 2, c0:c0 + cw]
            return out_tiles[et][:, c0:c0 + cw]

        def relu_epilogue(ps, bias_sb, out_tiles, et, c0, cw, on_dve):
            if on_dve:
                # (psum + bias) max 0 -- exact relu+bias as one DVE op
                nc.vector.tensor_scalar(
                    out=_eslice(out_tiles, et, c0, cw),
                    in0=ps,
                    scalar1=bias_sb[:, et:et + 1],
                    scalar2=0.0,
                    op0=mybir.AluOpType.add,
                    op1=mybir.AluOpType.max,
                )
            else:
                nc.scalar.activation(
                    out=_eslice(out_tiles, et, c0, cw),
                    in_=ps,
                    func=AF.Relu,
                    bias=bias_sb[:, et:et + 1],
                    scale=1.0,
                )

        def proj(xin, w_sb, bias_sb, out_tiles, chunks):
            # out_tiles[et] = relu(W[:, et].T @ x + b)
            n = 0
            for et in range(NET):
                for (c0, cw) in chunks:
                    ps = ppsum.tile([P, 512], F32, tag="proj")
                    for dt_ in range(NDT):
                        nc.tensor.matmul(
                            ps[:, 0:cw],
                            lhsT=w_sb[dt_][:, et * P:(et + 1) * P],
                            rhs=xin[dt_][:, c0:c0 + cw],
                            start=(dt_ == 0),
                            stop=(dt_ == NDT - 1),
                        )
                    # chains 0-2 on ACT: at an S->proj transition the DVE
                    # queue still drains the prior batch's po-scales, and
                    # chain n's first matmul waits on chain n-3's epilogue
                    relu_epilogue(ps[:, 0:cw], bias_sb, out_tiles, et, c0, cw,
                                  on_dve=(n % 2 == 1 and n >= 3))
                    n += 1

        def proj_coldstart(xin, w_sb, bias_sb, out_tiles, chunks, pfx):
            # Batch-0 projections: dt-major order so the PE consumes
            # (w_dt, x_dt) DMA pairs in arrival order; all NET*len(chunks)
            # accumulation chains are open at once, borrowing the (still
            # idle) S-phase psum pool.  Chain -> single-bank psum region:
            #   chunk0 (512 wide) x4 et -> spsum tiles 0,1 (two banks each)
            #   chunk1 (<=128)    x4 et -> ppsum x3 + cpsum
            sp0 = spsum.tile([P, SPAD], F32, tag="S", name=f"{pfx}c0a")
            sp1 = spsum.tile([P, SPAD], F32, tag="S", name=f"{pfx}c0b")
            big = [sp0[:, 0:512], sp0[:, 512:1024],
                   sp1[:, 0:512], sp1[:, 512:1024]]
            regions = {}
            for et in range(NET):
                regions[(et, 0)] = big[et]
            if len(chunks) > 1:
                cw1 = chunks[1][1]
                pps = [ppsum.tile([P, 512], F32, tag="proj",
                                  name=f"{pfx}c1{i}") for i in range(3)]
                pps.append(cpsum.tile([P, 512], F32, tag="cold",
                                      name=f"{pfx}c1x"))
                for et in range(NET):
                    regions[(et, 1)] = pps[et][:, 0:cw1]
            for dt_ in range(NDT):
                for et in range(NET):
                    for ci, (c0, cw) in enumerate(chunks):
                        nc.tensor.matmul(
                            regions[(et, ci)],
                            lhsT=w_sb[dt_][:, et * P:(et + 1) * P],
                            rhs=xin[dt_][:, c0:c0 + cw],
                            start=(dt_ == 0),
                            stop=(dt_ == NDT - 1),
                        )
            # chunk-major epilogues: S block 0 needs cols 0:128 of every et
            # tile, which chunk 0 covers -- drain those four chains first
            n = 0
            for ci, (c0, cw) in enumerate(chunks):
                for et in range(NET):
                    relu_epilogue(regions[(et, ci)], bias_sb, out_tiles,
                                  et, c0, cw, on_dve=(n % 2 == 1))
                    n += 1

        def mask_add(kraw, mask_sb, b):
            kTm = [actpool.tile([P, NKP], BF16, tag=f"kTm{et}",
                                name=f"kTm{et}_{b}")
                   for et in range(NET)]
            for et in range(NET):
                # split across gpsimd and vector so neither gates the S phase
                eng = nc.gpsimd if et % 2 == 0 else nc.vector
                eng.tensor_add(kTm[et][:], kraw[et][:], mask_sb[:])
            return kTm

        def s_stats(rs, pad_sb, rows=P):
            # row-sum -> subtract pad-column contribution -> reciprocal
            # (all on DVE: a cross-engine sub->recip chain measurably
            # stalls DVE head-of-line behind GpSimd's store issues)
            rsv = stpool.tile([P, 1], F32, tag="rsv")
            nc.vector.tensor_tensor(
                out=rsv[0:rows, :], in0=rs[0:rows, :], in1=pad_sb[0:rows, :],
                op=mybir.AluOpType.subtract,
            )
            rc = stpool.tile([P, 1], F32, tag="recip")
            nc.vector.reciprocal(out=rc[0:rows, :], in_=rsv[0:rows, :])
            return rc

        def s_block(b, ib, qTt, kTm, pad_sb):
            rows = rows_of(ib)
            sp = spsum.tile([P, SPAD], F32, tag="S")
            for (c0, cw) in kchunks:
                if s_fp8:
                    for j in range(NET // 2):
                        nc.tensor.matmul(
                            sp[0:rows, c0:c0 + cw],
                            lhsT=qTt[j][:, 0:2, ib * P:ib * P + rows],
                            rhs=kTm[j][:, 0:2, c0:c0 + cw],
                            start=(j == 0),
                            stop=(j == NET // 2 - 1),
                            perf_mode=mybir.MatmulPerfMode.DoubleRow,
                        )
                else:
                    for et in range(NET):
                        nc.tensor.matmul(
                            sp[0:rows, c0:c0 + cw],
                            lhsT=qTt[et][:, ib * P:ib * P + rows],
                            rhs=kTm[et][:, c0:c0 + cw],
                            start=(et == 0),
                            stop=(et == NET - 1),
                        )
            ex = epool.tile([P, NKP], BF16, tag="exp")
            rs = stpool.tile([P, 1], F32, tag="rowsum")
            nc.scalar.activation(
                out=ex[0:rows, :], in_=sp[0:rows, 0:NKP], func=AF.Exp,
                scale=SCALE, accum_out=rs[0:rows, :],
            )
            rc = s_stats(rs, pad_sb, rows)
            po = opool.tile([P, NKP], BF16, tag="po")
            nc.vector.tensor_scalar(
                out=po[0:rows, :], in0=ex[0:rows, :],
                scalar1=rc[0:rows, :], scalar2=None,
                op0=mybir.AluOpType.mult,
            )
            # alternate store queues so the output backlog drains 2x faster
            # (sync, not scalar: scalar's ACT must not stall behind DMA issue)
            eng = nc.gpsimd if ib % 2 == 0 else nc.sync
            eng.dma_start(out=out[b, ib * P:ib * P + rows, :],
                          in_=po[0:rows, :])

        def s_block_final(b, ib, qTt, kTm, pad_sb):
            # Last block of the kernel: chunk-major matmuls into separate
            # 1-bank psums + a fully split epilogue so the first chunk's
            # exp/mul/store overlap the second chunk's matmuls and exp --
            # shortening the serial tail after the last MM.
            rows = rows_of(ib)
            nch = len(kchunks)
            sps, rss, exs = [], [], []
            for ci, (c0, cw) in enumerate(kchunks):
                sps.append(ppsum.tile([P, 512], F32, tag="proj",
                                      name=f"fsp{ci}"))
                rss.append(stpool.tile([P, 1], F32, tag=f"rowsum{ci}",
                                       name=f"frs{ci}"))
                exs.append(epool.tile([P, cw], BF16, tag=f"fex{ci}",
                                      name=f"fex{ci}"))
            for ci, (c0, cw) in enumerate(kchunks):
                if s_fp8:
                    for j in range(NET // 2):
                        nc.tensor.matmul(
                            sps[ci][0:rows, 0:cw],
                            lhsT=qTt[j][:, 0:2, ib * P:ib * P + rows],
                            rhs=kTm[j][:, 0:2, c0:c0 + cw],
                            start=(j == 0),
                            stop=(j == NET // 2 - 1),
                            perf_mode=mybir.MatmulPerfMode.DoubleRow,
                        )
                else:
                    for et in range(NET):
                        nc.tensor.matmul(
                            sps[ci][0:rows, 0:cw],
                            lhsT=qTt[et][:, ib * P:ib * P + rows],
                            rhs=kTm[et][:, c0:c0 + cw],
                            start=(et == 0),
                            stop=(et == NET - 1),
                        )
                nc.scalar.activation(
                    out=exs[ci][0:rows, :], in_=sps[ci][0:rows, 0:cw],
                    func=AF.Exp, scale=SCALE, accum_out=rss[ci][0:rows, :],
                )
            rs = rss[0]
            for ci in range(1, nch):
                rst = stpool.tile([P, 1], F32, tag="rowsumt", name=f"frt{ci}")
                nc.vector.tensor_tensor(
                    out=rst[0:rows, :], in0=rs[0:rows, :],
                    in1=rss[ci][0:rows, :],
                    op=mybir.AluOpType.add)
                rs = rst
            rc = s_stats(rs, pad_sb, rows)
            for ci, (c0, cw) in enumerate(kchunks):
                poh = opool.tile([P, cw], BF16, tag=f"fpo{ci}", name=f"fpo{ci}")
                nc.vector.tensor_scalar(
                    out=poh[0:rows, :], in0=exs[ci][0:rows, :],
                    scalar1=rc[0:rows, :], scalar2=None,
                    op0=mybir.AluOpType.mult,
                )
                if cw > 256:
                    # split the store across two queues to drain the final
                    # transfer 2x faster (scalar's ACT work is done by now)
                    h = cw // 2
                    nc.sync.dma_start(
                        out=out[b, ib * P:ib * P + rows, c0:c0 + h],
                        in_=poh[0:rows, 0:h])
                    nc.scalar.dma_start(
                        out=out[b, ib * P:ib * P + rows, c0 + h:c0 + cw],
                        in_=poh[0:rows, h:cw])
                else:
                    nc.gpsimd.dma_start(
                        out=out[b, ib * P:ib * P + rows, c0:c0 + cw],
                        in_=poh[0:rows, :],
                    )

        def s_phase(b, qTt, kTm, pad_sb):
            for ib in range(NQB):
                if b == BL - 1 and ib == NQB - 1:
                    s_block_final(b, ib, qTt, kTm, pad_sb)
                else:
                    s_block(b, ib, qTt, kTm, pad_sb)

        cur = load_inputs(0)
        for b in range(BL):
            xk, xq, pad_sb, mask_sb = cur
            if use_mask:
                ktag = "kraw"
            else:
                ktag = "kTm"
            if s_fp8:
                kraw = [actpool.tile([P, 2, NKP], FP8, tag=f"{ktag}{j}",
                                     name=f"{ktag}{j}_{b}")
                        for j in range(NET // 2)]
            else:
                kraw = [actpool.tile([P, NKP], BF16, tag=f"{ktag}{et}",
                                     name=f"{ktag}{et}_{b}")
                        for et in range(NET)]
            if b == 0:
                proj_coldstart(xk, wk_sb, bk_sb, kraw, kchunks, pfx="coldk")
            else:
                proj(xk, wk_sb, bk_sb, kraw, kchunks)
            kTm = mask_add(kraw, mask_sb, b) if use_mask else kraw
            if s_fp8:
                qTt = [actpool.tile([P, 2, NQP], FP8, tag=f"qT{j}",
                                    name=f"qT{j}_{b}")
                       for j in range(NET // 2)]
            else:
                qTt = [actpool.tile([P, NQP], BF16, tag=f"qT{et}",
                                    name=f"qT{et}_{b}")
                       for et in range(NET)]
            if b == 0:
                proj_coldstart(xq, wq_sb, bq_sb, qTt, qchunks, pfx="coldq")
            else:
                proj(xq, wq_sb, bq_sb, qTt, qchunks)
            if b + 1 < BL:
                cur = load_inputs(b + 1)
            s_phase(b, qTt, kTm, pad_sb)


def _build(NQP, NKP, use_mask):
    nc = bacc.Bacc(
        "TRN2",
        target_bir_lowering=False,
        debug=False,
        enable_asserts=False,
        num_devices=NCORES,
    )
    qT = nc.dram_tensor("qT", [BL, D, NQP], BF16, kind="ExternalInput").ap()
    kT = nc.dram_tensor("kT", [BL, D, NKP], BF16, kind="ExternalInput").ap()
    Wq = nc.dram_tensor("Wq", [D, D], BF16, kind="ExternalInput").ap()
    Wk = nc.dram_tensor("Wk", [D, D], BF16, kind="ExternalInput").ap()
    bq = nc.dram_tensor("bq", [P, NET], F32, kind="ExternalInput").ap()
    bk = nc.dram_tensor("bk", [P, NET], F32, kind="ExternalInput").ap()
    padc = nc.dram_tensor("padc", [BL, P, 1], F32, kind="ExternalInput").ap()
    maskc = None
    if use_mask:
        maskc = nc.dram_tensor(
            "maskc", [BL, P, NKP], BF16, kind="ExternalInput").ap()
    out = nc.dram_tensor("out", [BL, NQP, NKP], BF16, kind="ExternalOutput").ap()

    with tile.TileContext(nc) as tc:
        _body(tc, qT, kT, Wq, Wk, bq, bk, padc, maskc, out, NQP, NKP)
    nc.compile()
    return nc


def _get_nc(NQP, NKP, use_mask):
    key = (NQP, NKP, use_mask)
    if key not in _CACHE:
        _CACHE[key] = _build(*key)
    return _CACHE[key]


def _pad64(n):
    # 64-col granularity: tail matmuls are free-dim-priced (no LDW floor),
    # so finer padding directly cuts PE cycles.  S-blocks still span 128
    # rows; a trailing 64-row block costs the same per column.
    return max(64, ((n + 63) // 64) * 64)


def _prep(query, key, query_mask, key_mask, Wq, bq, Wk, bk):
    bf = ml_dtypes.bfloat16
    query = np.asarray(query, dtype=np.float32)
    key = np.asarray(key, dtype=np.float32)
    qmask = np.asarray(query_mask) != 0
    kmask = np.asarray(key_mask) != 0
    qidx = [np.nonzero(qmask[g])[0] for g in range(B)]
    kidx = [np.nonzero(kmask[g])[0] for g in range(B)]
    NQP = _pad64(max(len(i) for i in qidx))
    NKP = _pad64(max(len(i) for i in kidx))
    use_mask = bool(np.any(np.asarray(bk, dtype=np.float32) != 0.0))

    Wq_b = np.asarray(Wq, dtype=np.float32).astype(bf)
    Wk_b = np.asarray(Wk, dtype=np.float32).astype(bf)
    # bias for feature e lives at partition e%128, column e//128
    bq_t = np.asarray(bq, dtype=np.float32).reshape(NET, P).T.copy()
    bk_t = np.asarray(bk, dtype=np.float32).reshape(NET, P).T.copy()

    in_maps = []
    for c in range(NCORES):
        qTc = np.zeros((BL, D, NQP), dtype=bf)
        kTc = np.zeros((BL, D, NKP), dtype=bf)
        padc = np.zeros((BL, P, 1), dtype=np.float32)
        imap = {"qT": qTc, "kT": kTc, "Wq": Wq_b, "Wk": Wk_b,
                "bq": bq_t, "bk": bk_t, "padc": padc}
        if use_mask:
            mk = np.zeros((BL, P, NKP), dtype=bf)
            imap["maskc"] = mk
        for b in range(BL):
            g = c * BL + b
            qi, ki = qidx[g], kidx[g]
            qTc[b, :, :len(qi)] = query[g][qi].T.astype(bf)
            kTc[b, :, :len(ki)] = key[g][ki].T.astype(bf)
            if use_mask:
                imap["maskc"][b, :, len(ki):] = bf(MASKC)
            else:
                padc[b, :, 0] = float(NKP - len(ki))
        in_maps.append(imap)
    return in_maps, qidx, kidx, NQP, NKP, use_mask


def run(query, key, query_mask, key_mask, Wq, bq, Wk, bk, **kwargs):
    """Run on hardware; returns (output, BassKernelResults)."""
    in_maps, qidx, kidx, NQP, NKP, use_mask = _prep(
        query, key, query_mask, key_mask, Wq, bq, Wk, bk)
    nc = _get_nc(NQP, NKP, use_mask)
    res = run_bass_kernel_spmd(nc, in_maps, core_ids=list(range(NCORES)),
                               **kwargs)
    full = np.zeros((B, LQ, LK), dtype=np.float32)
    for c in range(NCORES):
        oc = res.results[c]["out"]
        for b in range(BL):
            g = c * BL + b
            qi, ki = qidx[g], kidx[g]
            full[g][np.ix_(qi, ki)] = oc[b][:len(qi), :len(ki)].astype(np.float32)
    return full, res


def kernel(query, key, query_mask, key_mask, Wq, bq, Wk, bk):
    full, _ = run(query, key, query_mask, key_mask, Wq, bq, Wk, bk)
    return full


# revision 17
# speedup vs baseline: 1.0263x; 1.0263x over previous
"""Masked attention-weight kernel (dense_transformer) for 8 TRN2 NeuronCores.

Computes, for inputs query/key [32,1024,512] f32, masks [32,1024] i32:
    q = relu(query @ Wq + bq); k = relu(key @ Wk + bk)
    w = softmax((q @ k^T)/sqrt(512) + key_mask_additive) * query_mask
Output: [32, 1024, 1024] f32.

Strategy: data-parallel over batch (4 batches/core, no collectives) PLUS
host-side mask compaction.  Masked key columns have weight exactly 0 in the
reference (exp(-1e9) underflows) and masked query rows are zeroed, so the
host gathers only the valid ~512 query rows / key columns per batch, pads
them to a fixed NQP/NKP (multiple of 64, 576 for this data), and the device
runs dense attention on the compacted [NQP, NKP] problem -- ~2.4x fewer
matmul cycles than the full [1024,1024].  The host scatters the compact
bf16 output back into a zero-filled full-size f32 array.

Padded key columns are all-zero inputs, so (with zero bias -- true for this
problem) their projected features are 0, their logits are 0, and each
contributes exp(0)=1 to the softmax row-sum; the device subtracts the
host-provided pad count from the row-sum before taking the reciprocal.
If the key bias were nonzero the host instead ships an additive -1e4
column mask applied to the projected k (use_mask variant).

Per-core pipeline, per batch (all matmuls bf16 with f32 PSUM):
  1. kTm[e,j] = relu(Wk.T @ keyT + bk): PE matmuls in (512,64) psum-bank
     chunks -> relu+bias epilogue alternating ACT/DVE.
  2. qT[e,i] likewise.  Batch 0 runs both projections dt-major across 8
     open psum chains so the PE consumes (w_dt, x_dt) DMA pairs in arrival
     order at cold start.
  3. Per 128-row block: S = qT.T @ kTm (PE), ACT exp with fused row-sum,
     DVE pad-correction + reciprocal, DVE scale, DMA out (stores alternate
     between the gpsimd and sync queues).
"""

import sys

sys.path.insert(0, "/opt/trn_rl_repo")

import numpy as np
import ml_dtypes
from contextlib import ExitStack

import concourse.tile as tile
from concourse import bacc, mybir
from concourse.bass_utils import run_bass_kernel_spmd

P = 128
B, LQ, LK, D = 32, 1024, 1024, 512
NCORES = 8
BL = B // NCORES          # batches per core
NDT = D // P              # contraction tiles for projections
NET = D // P              # output-feature tiles (= S contraction tiles)
SCALE = float(1.0 / np.sqrt(D))
MASKC = -1.0e4

F32 = mybir.dt.float32
BF16 = mybir.dt.bfloat16
FP8 = mybir.dt.float8e4
AF = mybir.ActivationFunctionType

_CACHE = {}


def _chunks(width):
    """Split a free width into psum-bank-aligned chunks (<=512 each)."""
    out, c0 = [], 0
    while c0 < width:
        cw = min(512, width - c0)
        out.append((c0, cw))
        c0 += cw
    return out


def _body(tc, qT, kT, Wq, Wk, bq, bk, padc, maskc, out, NQP, NKP):
    nc = tc.nc
    # fp8 DoubleRow S-matmul measured L2 err 1.9e-2 vs the 2e-2 gate --
    # only ~3us faster than bf16 (S phase is ACT-bound), so keep bf16.
    s_fp8 = False
    NQB = (NQP + P - 1) // P  # S blocks per batch (last may be 64 rows)
    rows_of = lambda ib: min(P, NQP - ib * P)
    SPAD = ((NKP + 511) // 512) * 512   # psum tile width (bank aligned)
    kchunks = _chunks(NKP)
    qchunks = _chunks(NQP)
    use_mask = maskc is not None
    with ExitStack() as ctx:
        consts = ctx.enter_context(tc.tile_pool(name="consts", bufs=1))
        wpool = ctx.enter_context(tc.tile_pool(name="w", bufs=1))
        inpool = ctx.enter_context(tc.tile_pool(name="inp", bufs=2))
        actpool = ctx.enter_context(tc.tile_pool(name="act", bufs=2))
        mpool = ctx.enter_context(tc.tile_pool(name="mask", bufs=2))
        epool = ctx.enter_context(tc.tile_pool(name="exp", bufs=3))
        opool = ctx.enter_context(tc.tile_pool(name="pout", bufs=3))
        stpool = ctx.enter_context(tc.tile_pool(name="stat", bufs=6))
        ppsum = ctx.enter_context(tc.tile_pool(name="ppsum", bufs=3, space="PSUM"))
        spsum = ctx.enter_context(tc.tile_pool(name="spsum", bufs=2, space="PSUM"))
        cpsum = ctx.enter_context(tc.tile_pool(name="cpsum", bufs=1, space="PSUM"))

        # Weights on the scalar DMA queue, inputs on sync, small tensors on
        # gpsimd -- three queues pull concurrently at cold start.
        wk_sb = [wpool.tile([P, D], BF16, tag=f"wk{dt_}", name=f"wk{dt_}")
                 for dt_ in range(NDT)]
        wq_sb = [wpool.tile([P, D], BF16, tag=f"wq{dt_}", name=f"wq{dt_}")
                 for dt_ in range(NDT)]
        for dt_ in range(NDT):
            nc.scalar.dma_start(
                out=wk_sb[dt_][:], in_=Wk[dt_ * P:(dt_ + 1) * P, :])
        for dt_ in range(NDT):
            nc.scalar.dma_start(
                out=wq_sb[dt_][:], in_=Wq[dt_ * P:(dt_ + 1) * P, :])

        bk_sb = consts.tile([P, NET], F32)
        nc.gpsimd.dma_start(out=bk_sb[:], in_=bk[:])
        bq_sb = consts.tile([P, NET], F32)
        nc.gpsimd.dma_start(out=bq_sb[:], in_=bq[:])

        # PE warmup: 8 dummy matmuls (~3.4us of cold PE busy, exactly one
        # HAM activity window) on scratch tiles while the first input DMAs
        # are in flight, so the clock-gate reaches K=8/8 just before real
        # matmuls start. Results are never read.
        warm_in = consts.tile([P, 512], BF16, name="warm_in")
        nc.vector.memset(warm_in[:], 0.0)
        warm_ps = ppsum.tile([P, 512], F32, tag="proj", name="warm_ps")
        for _ in range(4):
            nc.tensor.matmul(
                warm_ps[:], lhsT=warm_in[:, 0:P], rhs=warm_in[:],
                start=True, stop=True,
            )

        def load_inputs(b):
            xk, xq = [], []
            for dt_ in range(NDT):
                t = inpool.tile([P, NKP], BF16, tag=f"xk{dt_}")
                if b == 0:
                    # split per chunk: the cold-start proj chain is gated by
                    # ARRIVAL of each chunk, not by descriptor issue count
                    for (c0, cw) in kchunks:
                        nc.sync.dma_start(
                            out=t[:, c0:c0 + cw],
                            in_=kT[b, dt_ * P:(dt_ + 1) * P, c0:c0 + cw])
                else:
                    nc.sync.dma_start(
                        out=t[:], in_=kT[b, dt_ * P:(dt_ + 1) * P, :])
                xk.append(t)
            for dt_ in range(NDT):
                t = inpool.tile([P, NQP], BF16, tag=f"xq{dt_}")
                if b == 0:
                    # gpsimd queue: nearly idle at cold start, so xq doesn't
                    # serialize behind the xk loads on sync
                    for (c0, cw) in qchunks:
                        nc.gpsimd.dma_start(
                            out=t[:, c0:c0 + cw],
                            in_=qT[b, dt_ * P:(dt_ + 1) * P, c0:c0 + cw])
                else:
                    nc.sync.dma_start(
                        out=t[:], in_=qT[b, dt_ * P:(dt_ + 1) * P, :])
                xq.append(t)
            pad_sb = mpool.tile([P, 1], F32, tag="padc")
            nc.gpsimd.dma_start(out=pad_sb[:], in_=padc[b])
            mask_sb = None
            if use_mask:
                mask_sb = mpool.tile([P, NKP], BF16, tag="maskc")
                nc.gpsimd.dma_start(out=mask_sb[:], in_=maskc[b])
            return xk, xq, pad_sb, mask_sb

        def _eslice(out_tiles, et, c0, cw):
            # fp8 mode packs et pairs into [P, 2, N] DoubleRow operand tiles
            if s_fp8:
                return out_tiles[et // 2][:, et % 2, c0:c0 + cw]
            return out_tiles[et][:, c0:c0 + cw]

        def relu_epilogue(ps, bias_sb, out_tiles, et, c0, cw, on_dve):
            if on_dve:
                # (psum + bias) max 0 -- exact relu+bias as one DVE op
                nc.vector.tensor_scalar(
                    out=_eslice(out_tiles, et, c0, cw),
                    in0=ps,
                    scalar1=bias_sb[:, et:et + 1],
                    scalar2=0.0,
                    op0=mybir.AluOpType.add,
                    op1=mybir.AluOpType.max,
                )
            else:
                nc.scalar.activation(
                    out=_eslice(out_tiles, et, c0, cw),
                    in_=ps,
                    func=AF.Relu,
                    bias=bias_sb[:, et:et + 1],
                    scale=1.0,
                )

        def proj(xin, w_sb, bias_sb, out_tiles, chunks):
            # out_tiles[et] = relu(W[:, et].T @ x + b)
            n = 0
            for et in range(NET):
                for (c0, cw) in chunks:
                    ps = ppsum.tile([P, 512], F32, tag="proj")
                    for dt_ in range(NDT):
                        nc.tensor.matmul(
                            ps[:, 0:cw],
                            lhsT=w_sb[dt_][:, et * P:(et + 1) * P],
                            rhs=xin[dt_][:, c0:c0 + cw],
                            start=(dt_ == 0),
                            stop=(dt_ == NDT - 1),
                        )
                    relu_epilogue(ps[:, 0:cw], bias_sb, out_tiles, et, c0, cw,
                                  on_dve=(n % 2 == 1))
                    n += 1

        def proj_coldstart(xin, w_sb, bias_sb, out_tiles, chunks, pfx):
            # Batch-0 projections: dt-major order so the PE consumes
            # (w_dt, x_dt) DMA pairs in arrival order; all NET*len(chunks)
            # accumulation chains are open at once, borrowing the (still
            # idle) S-phase psum pool.  Chain -> single-bank psum region:
            #   chunk0 (512 wide) x4 et -> spsum tiles 0,1 (two banks each)
            #   chunk1 (<=128)    x4 et -> ppsum x3 + cpsum
            sp0 = spsum.tile([P, SPAD], F32, tag="S", name=f"{pfx}c0a")
            sp1 = spsum.tile([P, SPAD], F32, tag="S", name=f"{pfx}c0b")
            big = [sp0[:, 0:512], sp0[:, 512:1024],
                   sp1[:, 0:512], sp1[:, 512:1024]]
            regions = {}
            for et in range(NET):
                regions[(et, 0)] = big[et]
            if len(chunks) > 1:
                cw1 = chunks[1][1]
                pps = [ppsum.tile([P, 512], F32, tag="proj",
                                  name=f"{pfx}c1{i}") for i in range(3)]
                pps.append(cpsum.tile([P, 512], F32, tag="cold",
                                      name=f"{pfx}c1x"))
                for et in range(NET):
                    regions[(et, 1)] = pps[et][:, 0:cw1]
            for dt_ in range(NDT):
                for et in range(NET):
                    for ci, (c0, cw) in enumerate(chunks):
                        nc.tensor.matmul(
                            regions[(et, ci)],
                            lhsT=w_sb[dt_][:, et * P:(et + 1) * P],
                            rhs=xin[dt_][:, c0:c0 + cw],
                            start=(dt_ == 0),
                            stop=(dt_ == NDT - 1),
                        )
            # chunk-major epilogues: S block 0 needs cols 0:128 of every et
            # tile, which chunk 0 covers -- drain those four chains first
            n = 0
            for ci, (c0, cw) in enumerate(chunks):
                for et in range(NET):
                    relu_epilogue(regions[(et, ci)], bias_sb, out_tiles,
                                  et, c0, cw, on_dve=(n % 2 == 1))
                    n += 1

        def mask_add(kraw, mask_sb, b):
            kTm = [actpool.tile([P, NKP], BF16, tag=f"kTm{et}",
                                name=f"kTm{et}_{b}")
                   for et in range(NET)]
            for et in range(NET):
                # split across gpsimd and vector so neither gates the S phase
                eng = nc.gpsimd if et % 2 == 0 else nc.vector
                eng.tensor_add(kTm[et][:], kraw[et][:], mask_sb[:])
            return kTm

        def s_stats(rs, pad_sb, rows=P):
            # row-sum -> subtract pad-column contribution -> reciprocal
            # (all on DVE: a cross-engine sub->recip chain measurably
            # stalls DVE head-of-line behind GpSimd's store issues)
            rsv = stpool.tile([P, 1], F32, tag="rsv")
            nc.vector.tensor_tensor(
                out=rsv[0:rows, :], in0=rs[0:rows, :], in1=pad_sb[0:rows, :],
                op=mybir.AluOpType.subtract,
            )
            rc = stpool.tile([P, 1], F32, tag="recip")
            nc.vector.reciprocal(out=rc[0:rows, :], in_=rsv[0:rows, :])
            return rc

        def s_block(b, ib, qTt, kTm, pad_sb):
            rows = rows_of(ib)
            sp = spsum.tile([P, SPAD], F32, tag="S")
            for (c0, cw) in kchunks:
                if s_fp8:
                    for j in range(NET // 2):
                        nc.tensor.matmul(
                            sp[0:rows, c0:c0 + cw],
                            lhsT=qTt[j][:, 0:2, ib * P:ib * P + rows],
                            rhs=kTm[j][:, 0:2, c0:c0 + cw],
                            start=(j == 0),
                            stop=(j == NET // 2 - 1),
                            perf_mode=mybir.MatmulPerfMode.DoubleRow,
                        )
                else:
                    for et in range(NET):
                        nc.tensor.matmul(
                            sp[0:rows, c0:c0 + cw],
                            lhsT=qTt[et][:, ib * P:ib * P + rows],
                            rhs=kTm[et][:, c0:c0 + cw],
                            start=(et == 0),
                            stop=(et == NET - 1),
                        )
            ex = epool.tile([P, NKP], BF16, tag="exp")
            rs = stpool.tile([P, 1], F32, tag="rowsum")
            nc.scalar.activation(
                out=ex[0:rows, :], in_=sp[0:rows, 0:NKP], func=AF.Exp,
                scale=SCALE, accum_out=rs[0:rows, :],
            )
            rc = s_stats(rs, pad_sb, rows)
            po = opool.tile([P, NKP], BF16, tag="po")
            nc.vector.tensor_scalar(
                out=po[0:rows, :], in0=ex[0:rows, :],
                scalar1=rc[0:rows, :], scalar2=None,
                op0=mybir.AluOpType.mult,
            )
            # alternate store queues so the output backlog drains 2x faster
            # (sync, not scalar: scalar's ACT must not stall behind DMA issue)
            eng = nc.gpsimd if ib % 2 == 0 else nc.sync
            eng.dma_start(out=out[b, ib * P:ib * P + rows, :],
                          in_=po[0:rows, :])

        def s_block_final(b, ib, qTt, kTm, pad_sb):
            # Last block of the kernel: chunk-major matmuls into separate
            # 1-bank psums + a fully split epilogue so the first chunk's
            # exp/mul/store overlap the second chunk's matmuls and exp --
            # shortening the serial tail after the last MM.
            rows = rows_of(ib)
            nch = len(kchunks)
            sps, rss, exs = [], [], []
            for ci, (c0, cw) in enumerate(kchunks):
                sps.append(ppsum.tile([P, 512], F32, tag="proj",
                                      name=f"fsp{ci}"))
                rss.append(stpool.tile([P, 1], F32, tag=f"rowsum{ci}",
                                       name=f"frs{ci}"))
                exs.append(epool.tile([P, cw], BF16, tag=f"fex{ci}",
                                      name=f"fex{ci}"))
            for ci, (c0, cw) in enumerate(kchunks):
                if s_fp8:
                    for j in range(NET // 2):
                        nc.tensor.matmul(
                            sps[ci][0:rows, 0:cw],
                            lhsT=qTt[j][:, 0:2, ib * P:ib * P + rows],
                            rhs=kTm[j][:, 0:2, c0:c0 + cw],
                            start=(j == 0),
                            stop=(j == NET // 2 - 1),
                            perf_mode=mybir.MatmulPerfMode.DoubleRow,
                        )
                else:
                    for et in range(NET):
                        nc.tensor.matmul(
                            sps[ci][0:rows, 0:cw],
                            lhsT=qTt[et][:, ib * P:ib * P + rows],
                            rhs=kTm[et][:, c0:c0 + cw],
                            start=(et == 0),
                            stop=(et == NET - 1),
                        )
                nc.scalar.activation(
                    out=exs[ci][0:rows, :], in_=sps[ci][0:rows, 0:cw],
                    func=AF.Exp, scale=SCALE, accum_out=rss[ci][0:rows, :],
                )
            rs = rss[0]
            for ci in range(1, nch):
                rst = stpool.tile([P, 1], F32, tag="rowsumt", name=f"frt{ci}")
                nc.vector.tensor_tensor(
                    out=rst[0:rows, :], in0=rs[0:rows, :],
                    in1=rss[ci][0:rows, :],
                    op=mybir.AluOpType.add)
                rs = rst
            rc = s_stats(rs, pad_sb, rows)
            for ci, (c0, cw) in enumerate(kchunks):
                poh = opool.tile([P, cw], BF16, tag=f"fpo{ci}", name=f"fpo{ci}")
                nc.vector.tensor_scalar(
                    out=poh[0:rows, :], in0=exs[ci][0:rows, :],
                    scalar1=rc[0:rows, :], scalar2=None,
                    op0=mybir.AluOpType.mult,
                )
                if cw > 256:
                    # split the store across two queues to drain the final
                    # transfer 2x faster (scalar's ACT work is done by now)
                    h = cw // 2
                    nc.sync.dma_start(
                        out=out[b, ib * P:ib * P + rows, c0:c0 + h],
                        in_=poh[0:rows, 0:h])
                    nc.scalar.dma_start(
                        out=out[b, ib * P:ib * P + rows, c0 + h:c0 + cw],
                        in_=poh[0:rows, h:cw])
                else:
                    nc.gpsimd.dma_start(
                        out=out[b, ib * P:ib * P + rows, c0:c0 + cw],
                        in_=poh[0:rows, :],
                    )

        def s_phase(b, qTt, kTm, pad_sb):
            for ib in range(NQB):
                if b == BL - 1 and ib == NQB - 1:
                    s_block_final(b, ib, qTt, kTm, pad_sb)
                else:
                    s_block(b, ib, qTt, kTm, pad_sb)

        cur = load_inputs(0)
        for b in range(BL):
            xk, xq, pad_sb, mask_sb = cur
            if use_mask:
                ktag = "kraw"
            else:
                ktag = "kTm"
            if s_fp8:
                kraw = [actpool.tile([P, 2, NKP], FP8, tag=f"{ktag}{j}",
                                     name=f"{ktag}{j}_{b}")
                        for j in range(NET // 2)]
            else:
                kraw = [actpool.tile([P, NKP], BF16, tag=f"{ktag}{et}",
                                     name=f"{ktag}{et}_{b}")
                        for et in range(NET)]
            if b == 0:
                proj_coldstart(xk, wk_sb, bk_sb, kraw, kchunks, pfx="coldk")
            else:
                proj(xk, wk_sb, bk_sb, kraw, kchunks)
            kTm = mask_add(kraw, mask_sb, b) if use_mask else kraw
            if s_fp8:
                qTt = [actpool.tile([P, 2, NQP], FP8, tag=f"qT{j}",
                                    name=f"qT{j}_{b}")
                       for j in range(NET // 2)]
            else:
                qTt = [actpool.tile([P, NQP], BF16, tag=f"qT{et}",
                                    name=f"qT{et}_{b}")
                       for et in range(NET)]
            if b == 0:
                proj_coldstart(xq, wq_sb, bq_sb, qTt, qchunks, pfx="coldq")
            else:
                proj(xq, wq_sb, bq_sb, qTt, qchunks)
            if b + 1 < BL:
                cur = load_inputs(b + 1)
            s_phase(b, qTt, kTm, pad_sb)


def _build(NQP, NKP, use_mask):
    nc = bacc.Bacc(
        "TRN2",
        target_bir_lowering=False,
        debug=False,
        enable_asserts=False,
        num_devices=NCORES,
    )
    qT = nc.dram_tensor("qT", [BL, D, NQP], BF16, kind="ExternalInput").ap()
    kT = nc.dram_tensor("kT", [BL, D, NKP], BF16, kind="ExternalInput").ap()
    Wq = nc.dram_tensor("Wq", [D, D], BF16, kind="ExternalInput").ap()
    Wk = nc.dram_tensor("Wk", [D, D], BF16, kind="ExternalInput").ap()
    bq = nc.dram_tensor("bq", [P, NET], F32, kind="ExternalInput").ap()
    bk = nc.dram_tensor("bk", [P, NET], F32, kind="ExternalInput").ap()
    padc = nc.dram_tensor("padc", [BL, P, 1], F32, kind="ExternalInput").ap()
    maskc = None
    if use_mask:
        maskc = nc.dram_tensor(
            "maskc", [BL, P, NKP], BF16, kind="ExternalInput").ap()
    out = nc.dram_tensor("out", [BL, NQP, NKP], BF16, kind="ExternalOutput").ap()

    with tile.TileContext(nc) as tc:
        _body(tc, qT, kT, Wq, Wk, bq, bk, padc, maskc, out, NQP, NKP)
    nc.compile()
    return nc


def _get_nc(NQP, NKP, use_mask):
    key = (NQP, NKP, use_mask)
    if key not in _CACHE:
        _CACHE[key] = _build(*key)
    return _CACHE[key]


def _pad64(n):
    # 64-col granularity: tail matmuls are free-dim-priced (no LDW floor),
    # so finer padding directly cuts PE cycles.  S-blocks still span 128
    # rows; a trailing 64-row block costs the same per column.
    return max(64, ((n + 63) // 64) * 64)


def _prep(query, key, query_mask, key_mask, Wq, bq, Wk, bk):
    bf = ml_dtypes.bfloat16
    query = np.asarray(query, dtype=np.float32)
    key = np.asarray(key, dtype=np.float32)
    qmask = np.asarray(query_mask) != 0
    kmask = np.asarray(key_mask) != 0
    qidx = [np.nonzero(qmask[g])[0] for g in range(B)]
    kidx = [np.nonzero(kmask[g])[0] for g in range(B)]
    NQP = _pad64(max(len(i) for i in qidx))
    NKP = _pad64(max(len(i) for i in kidx))
    use_mask = bool(np.any(np.asarray(bk, dtype=np.float32) != 0.0))

    Wq_b = np.asarray(Wq, dtype=np.float32).astype(bf)
    Wk_b = np.asarray(Wk, dtype=np.float32).astype(bf)
    # bias for feature e lives at partition e%128, column e//128
    bq_t = np.asarray(bq, dtype=np.float32).reshape(NET, P).T.copy()
    bk_t = np.asarray(bk, dtype=np.float32).reshape(NET, P).T.copy()

    in_maps = []
    for c in range(NCORES):
        qTc = np.zeros((BL, D, NQP), dtype=bf)
        kTc = np.zeros((BL, D, NKP), dtype=bf)
        padc = np.zeros((BL, P, 1), dtype=np.float32)
        imap = {"qT": qTc, "kT": kTc, "Wq": Wq_b, "Wk": Wk_b,
                "bq": bq_t, "bk": bk_t, "padc": padc}
        if use_mask:
            mk = np.zeros((BL, P, NKP), dtype=bf)
            imap["maskc"] = mk
        for b in range(BL):
            g = c * BL + b
            qi, ki = qidx[g], kidx[g]
            qTc[b, :, :len(qi)] = query[g][qi].T.astype(bf)
            kTc[b, :, :len(ki)] = key[g][ki].T.astype(bf)
            if use_mask:
                imap["maskc"][b, :, len(ki):] = bf(MASKC)
            else:
                padc[b, :, 0] = float(NKP - len(ki))
        in_maps.append(imap)
    return in_maps, qidx, kidx, NQP, NKP, use_mask


def run(query, key, query_mask, key_mask, Wq, bq, Wk, bk, **kwargs):
    """Run on hardware; returns (output, BassKernelResults)."""
    in_maps, qidx, kidx, NQP, NKP, use_mask = _prep(
        query, key, query_mask, key_mask, Wq, bq, Wk, bk)
    nc = _get_nc(NQP, NKP, use_mask)
    res = run_bass_kernel_spmd(nc, in_maps, core_ids=list(range(NCORES)),
                               **kwargs)
    full = np.zeros((B, LQ, LK), dtype=np.float32)
    for c in range(NCORES):
        oc = res.results[c]["out"]
        for b in range(BL):
            g = c * BL + b
            qi, ki = qidx[g], kidx[g]
            full[g][np.ix_(qi, ki)] = oc[b][:len(qi), :len(ki)].astype(np.float32)
    return full, res


def kernel(query, key, query_mask, key_mask, Wq, bq, Wk, bk):
    full, _ = run(query, key, query_mask, key_mask, Wq, bq, Wk, bk)
    return full


# revision 18
# speedup vs baseline: 1.0493x; 1.0223x over previous
"""Masked attention-weight kernel (dense_transformer) for 8 TRN2 NeuronCores.

Computes, for inputs query/key [32,1024,512] f32, masks [32,1024] i32:
    q = relu(query @ Wq + bq); k = relu(key @ Wk + bk)
    w = softmax((q @ k^T)/sqrt(512) + key_mask_additive) * query_mask
Output: [32, 1024, 1024] f32.

Strategy: data-parallel over batch (4 batches/core, no collectives) PLUS
host-side mask compaction.  Masked key columns have weight exactly 0 in the
reference (exp(-1e9) underflows) and masked query rows are zeroed, so the
host gathers only the valid ~512 query rows / key columns per batch, pads
them to a fixed NQP/NKP (multiple of 64, 576 for this data), and the device
runs dense attention on the compacted [NQP, NKP] problem -- ~2.4x fewer
matmul cycles than the full [1024,1024].  The host scatters the compact
bf16 output back into a zero-filled full-size f32 array.

Padded key columns are all-zero inputs, so (with zero bias -- true for this
problem) their projected features are 0, their logits are 0, and each
contributes exp(0)=1 to the softmax row-sum; the device subtracts the
host-provided pad count from the row-sum before taking the reciprocal.
If the key bias were nonzero the host instead ships an additive -1e4
column mask applied to the projected k (use_mask variant).

Per-core pipeline, per batch (all matmuls bf16 with f32 PSUM):
  1. kTm[e,j] = relu(Wk.T @ keyT + bk): PE matmuls in (512,64) psum-bank
     chunks -> relu+bias epilogue alternating ACT/DVE.
  2. qT[e,i] likewise.  Batch 0 runs both projections dt-major across 8
     open psum chains so the PE consumes (w_dt, x_dt) DMA pairs in arrival
     order at cold start.
  3. Per 128-row block: S = qT.T @ kTm (PE), ACT exp with fused row-sum,
     DVE pad-correction + reciprocal, DVE scale, DMA out (stores alternate
     between the gpsimd and sync queues).
"""

import sys

sys.path.insert(0, "/opt/trn_rl_repo")

import numpy as np
import ml_dtypes
from contextlib import ExitStack

import concourse.tile as tile
from concourse import bacc, mybir
from concourse.bass_utils import run_bass_kernel_spmd

P = 128
B, LQ, LK, D = 32, 1024, 1024, 512
NCORES = 8
BL = B // NCORES          # batches per core
NDT = D // P              # contraction tiles for projections
NET = D // P              # output-feature tiles (= S contraction tiles)
SCALE = float(1.0 / np.sqrt(D))
MASKC = -1.0e4

F32 = mybir.dt.float32
BF16 = mybir.dt.bfloat16
FP8 = mybir.dt.float8e4
AF = mybir.ActivationFunctionType

_CACHE = {}


def _chunks(width):
    """Split a free width into psum-bank-aligned chunks (<=512 each)."""
    out, c0 = [], 0
    while c0 < width:
        cw = min(512, width - c0)
        out.append((c0, cw))
        c0 += cw
    return out


def _body(tc, qT, kT, Wq, Wk, bq, bk, padc, maskc, out, NQP, NKP):
    nc = tc.nc
    # fp8 DoubleRow S-matmul measured L2 err 1.9e-2 vs the 2e-2 gate --
    # only ~3us faster than bf16 (S phase is ACT-bound), so keep bf16.
    s_fp8 = False
    NQB = (NQP + P - 1) // P  # S blocks per batch (last may be 64 rows)
    rows_of = lambda ib: min(P, NQP - ib * P)
    SPAD = ((NKP + 511) // 512) * 512   # psum tile width (bank aligned)
    kchunks = _chunks(NKP)
    qchunks = _chunks(NQP)
    use_mask = maskc is not None
    with ExitStack() as ctx:
        consts = ctx.enter_context(tc.tile_pool(name="consts", bufs=1))
        wpool = ctx.enter_context(tc.tile_pool(name="w", bufs=1))
        inpool = ctx.enter_context(tc.tile_pool(name="inp", bufs=2))
        actpool = ctx.enter_context(tc.tile_pool(name="act", bufs=2))
        mpool = ctx.enter_context(tc.tile_pool(name="mask", bufs=2))
        epool = ctx.enter_context(tc.tile_pool(name="exp", bufs=3))
        opool = ctx.enter_context(tc.tile_pool(name="pout", bufs=3))
        stpool = ctx.enter_context(tc.tile_pool(name="stat", bufs=6))
        ppsum = ctx.enter_context(tc.tile_pool(name="ppsum", bufs=3, space="PSUM"))
        spsum = ctx.enter_context(tc.tile_pool(name="spsum", bufs=2, space="PSUM"))
        cpsum = ctx.enter_context(tc.tile_pool(name="cpsum", bufs=1, space="PSUM"))

        # Weights on the scalar DMA queue, inputs on sync, small tensors on
        # gpsimd -- three queues pull concurrently at cold start.
        wk_sb = [wpool.tile([P, D], BF16, tag=f"wk{dt_}", name=f"wk{dt_}")
                 for dt_ in range(NDT)]
        wq_sb = [wpool.tile([P, D], BF16, tag=f"wq{dt_}", name=f"wq{dt_}")
                 for dt_ in range(NDT)]
        for dt_ in range(NDT):
            nc.scalar.dma_start(
                out=wk_sb[dt_][:], in_=Wk[dt_ * P:(dt_ + 1) * P, :])
        for dt_ in range(NDT):
            nc.scalar.dma_start(
                out=wq_sb[dt_][:], in_=Wq[dt_ * P:(dt_ + 1) * P, :])

        bk_sb = consts.tile([P, NET], F32)
        nc.gpsimd.dma_start(out=bk_sb[:], in_=bk[:])
        bq_sb = consts.tile([P, NET], F32)
        nc.gpsimd.dma_start(out=bq_sb[:], in_=bq[:])

        # PE warmup: 8 dummy matmuls (~3.4us of cold PE busy, exactly one
        # HAM activity window) on scratch tiles while the first input DMAs
        # are in flight, so the clock-gate reaches K=8/8 just before real
        # matmuls start. Results are never read.
        warm_in = consts.tile([P, 512], BF16, name="warm_in")
        nc.vector.memset(warm_in[:], 0.0)
        warm_ps = ppsum.tile([P, 512], F32, tag="proj", name="warm_ps")
        for _ in range(4):
            nc.tensor.matmul(
                warm_ps[:], lhsT=warm_in[:, 0:P], rhs=warm_in[:],
                start=True, stop=True,
            )

        def load_inputs(b):
            xk, xq = [], []
            for dt_ in range(NDT):
                t = inpool.tile([P, NKP], BF16, tag=f"xk{dt_}")
                if b == 0:
                    # split per chunk: the cold-start proj chain is gated by
                    # ARRIVAL of each chunk, not by descriptor issue count
                    for (c0, cw) in kchunks:
                        nc.sync.dma_start(
                            out=t[:, c0:c0 + cw],
                            in_=kT[b, dt_ * P:(dt_ + 1) * P, c0:c0 + cw])
                else:
                    nc.sync.dma_start(
                        out=t[:], in_=kT[b, dt_ * P:(dt_ + 1) * P, :])
                xk.append(t)
            for dt_ in range(NDT):
                t = inpool.tile([P, NQP], BF16, tag=f"xq{dt_}")
                if b == 0:
                    # gpsimd queue: nearly idle at cold start, so xq doesn't
                    # serialize behind the xk loads on sync
                    for (c0, cw) in qchunks:
                        nc.gpsimd.dma_start(
                            out=t[:, c0:c0 + cw],
                            in_=qT[b, dt_ * P:(dt_ + 1) * P, c0:c0 + cw])
                else:
                    nc.sync.dma_start(
                        out=t[:], in_=qT[b, dt_ * P:(dt_ + 1) * P, :])
                xq.append(t)
            pad_sb = mpool.tile([P, 1], F32, tag="padc")
            nc.gpsimd.dma_start(out=pad_sb[:], in_=padc[b])
            mask_sb = None
            if use_mask:
                mask_sb = mpool.tile([P, NKP], BF16, tag="maskc")
                nc.gpsimd.dma_start(out=mask_sb[:], in_=maskc[b])
            return xk, xq, pad_sb, mask_sb

        def _eslice(out_tiles, et, c0, cw):
            # fp8 mode packs et pairs into [P, 2, N] DoubleRow operand tiles
            if s_fp8:
                return out_tiles[et // 2][:, et % 2, c0:c0 + cw]
            return out_tiles[et][:, c0:c0 + cw]

        def relu_epilogue(ps, bias_sb, out_tiles, et, c0, cw, on_dve):
            if on_dve:
                # (psum + bias) max 0 -- exact relu+bias as one DVE op
                nc.vector.tensor_scalar(
                    out=_eslice(out_tiles, et, c0, cw),
                    in0=ps,
                    scalar1=bias_sb[:, et:et + 1],
                    scalar2=0.0,
                    op0=mybir.AluOpType.add,
                    op1=mybir.AluOpType.max,
                )
            else:
                nc.scalar.activation(
                    out=_eslice(out_tiles, et, c0, cw),
                    in_=ps,
                    func=AF.Relu,
                    bias=bias_sb[:, et:et + 1],
                    scale=1.0,
                )

        def proj(xin, w_sb, bias_sb, out_tiles, chunks):
            # out_tiles[et] = relu(W[:, et].T @ x + b)
            n = 0
            for et in range(NET):
                for (c0, cw) in chunks:
                    ps = ppsum.tile([P, 512], F32, tag="proj")
                    for dt_ in range(NDT):
                        nc.tensor.matmul(
                            ps[:, 0:cw],
                            lhsT=w_sb[dt_][:, et * P:(et + 1) * P],
                            rhs=xin[dt_][:, c0:c0 + cw],
                            start=(dt_ == 0),
                            stop=(dt_ == NDT - 1),
                        )
                    relu_epilogue(ps[:, 0:cw], bias_sb, out_tiles, et, c0, cw,
                                  on_dve=(n % 2 == 1))
                    n += 1

        def proj_coldstart(xin, w_sb, bias_sb, out_tiles, chunks, pfx):
            # Batch-0 projections: dt-major order so the PE consumes
            # (w_dt, x_dt) DMA pairs in arrival order; all NET*len(chunks)
            # accumulation chains are open at once, borrowing the (still
            # idle) S-phase psum pool.  Chain -> single-bank psum region:
            #   chunk0 (512 wide) x4 et -> spsum tiles 0,1 (two banks each)
            #   chunk1 (<=128)    x4 et -> ppsum x3 + cpsum
            sp0 = spsum.tile([P, SPAD], F32, tag="S", name=f"{pfx}c0a")
            sp1 = spsum.tile([P, SPAD], F32, tag="S", name=f"{pfx}c0b")
            big = [sp0[:, 0:512], sp0[:, 512:1024],
                   sp1[:, 0:512], sp1[:, 512:1024]]
            regions = {}
            for et in range(NET):
                regions[(et, 0)] = big[et]
            if len(chunks) > 1:
                cw1 = chunks[1][1]
                pps = [ppsum.tile([P, 512], F32, tag="proj",
                                  name=f"{pfx}c1{i}") for i in range(3)]
                pps.append(cpsum.tile([P, 512], F32, tag="cold",
                                      name=f"{pfx}c1x"))
                for et in range(NET):
                    regions[(et, 1)] = pps[et][:, 0:cw1]
            for dt_ in range(NDT):
                for et in range(NET):
                    for ci, (c0, cw) in enumerate(chunks):
                        nc.tensor.matmul(
                            regions[(et, ci)],
                            lhsT=w_sb[dt_][:, et * P:(et + 1) * P],
                            rhs=xin[dt_][:, c0:c0 + cw],
                            start=(dt_ == 0),
                            stop=(dt_ == NDT - 1),
                        )
            # chunk-major epilogues: S block 0 needs cols 0:128 of every et
            # tile, which chunk 0 covers -- drain those four chains first
            n = 0
            for ci, (c0, cw) in enumerate(chunks):
                for et in range(NET):
                    relu_epilogue(regions[(et, ci)], bias_sb, out_tiles,
                                  et, c0, cw, on_dve=(n % 2 == 1))
                    n += 1

        def mask_add(kraw, mask_sb, b):
            kTm = [actpool.tile([P, NKP], BF16, tag=f"kTm{et}",
                                name=f"kTm{et}_{b}")
                   for et in range(NET)]
            for et in range(NET):
                # split across gpsimd and vector so neither gates the S phase
                eng = nc.gpsimd if et % 2 == 0 else nc.vector
                eng.tensor_add(kTm[et][:], kraw[et][:], mask_sb[:])
            return kTm

        def s_stats(rs, pad_sb, rows=P):
            # row-sum -> subtract pad-column contribution -> reciprocal
            # (all on DVE: a cross-engine sub->recip chain measurably
            # stalls DVE head-of-line behind GpSimd's store issues)
            rsv = stpool.tile([P, 1], F32, tag="rsv")
            nc.vector.tensor_tensor(
                out=rsv[0:rows, :], in0=rs[0:rows, :], in1=pad_sb[0:rows, :],
                op=mybir.AluOpType.subtract,
            )
            rc = stpool.tile([P, 1], F32, tag="recip")
            nc.vector.reciprocal(out=rc[0:rows, :], in_=rsv[0:rows, :])
            return rc

        def s_block(b, ib, qTt, kTm, pad_sb):
            rows = rows_of(ib)
            sp = spsum.tile([P, SPAD], F32, tag="S")
            for (c0, cw) in kchunks:
                if s_fp8:
                    for j in range(NET // 2):
                        nc.tensor.matmul(
                            sp[0:rows, c0:c0 + cw],
                            lhsT=qTt[j][:, 0:2, ib * P:ib * P + rows],
                            rhs=kTm[j][:, 0:2, c0:c0 + cw],
                            start=(j == 0),
                            stop=(j == NET // 2 - 1),
                            perf_mode=mybir.MatmulPerfMode.DoubleRow,
                        )
                else:
                    for et in range(NET):
                        nc.tensor.matmul(
                            sp[0:rows, c0:c0 + cw],
                            lhsT=qTt[et][:, ib * P:ib * P + rows],
                            rhs=kTm[et][:, c0:c0 + cw],
                            start=(et == 0),
                            stop=(et == NET - 1),
                        )
            ex = epool.tile([P, NKP], BF16, tag="exp")
            rs = stpool.tile([P, 1], F32, tag="rowsum")
            nc.scalar.activation(
                out=ex[0:rows, :], in_=sp[0:rows, 0:NKP], func=AF.Exp,
                scale=SCALE, accum_out=rs[0:rows, :],
            )
            rc = s_stats(rs, pad_sb, rows)
            po = opool.tile([P, NKP], BF16, tag="po")
            nc.vector.tensor_scalar(
                out=po[0:rows, :], in0=ex[0:rows, :],
                scalar1=rc[0:rows, :], scalar2=None,
                op0=mybir.AluOpType.mult,
            )
            # alternate store queues so the output backlog drains 2x faster
            # (sync, not scalar: scalar's ACT must not stall behind DMA issue)
            eng = nc.gpsimd if ib % 2 == 0 else nc.sync
            eng.dma_start(out=out[b, ib * P:ib * P + rows, :],
                          in_=po[0:rows, :])

        def s_block_final(b, ib, qTt, kTm, pad_sb, last=True):
            # Last block of the kernel: chunk-major matmuls into separate
            # 1-bank psums + a fully split epilogue so the first chunk's
            # exp/mul/store overlap the second chunk's matmuls and exp --
            # shortening the serial tail after the last MM.
            rows = rows_of(ib)
            nch = len(kchunks)
            sps, rss, exs = [], [], []
            for ci, (c0, cw) in enumerate(kchunks):
                sps.append(ppsum.tile([P, 512], F32, tag="proj",
                                      name=f"fsp{ci}"))
                rss.append(stpool.tile([P, 1], F32, tag=f"rowsum{ci}",
                                       name=f"frs{ci}"))
                exs.append(epool.tile([P, cw], BF16, tag=f"fex{ci}",
                                      name=f"fex{ci}"))
            for ci, (c0, cw) in enumerate(kchunks):
                if s_fp8:
                    for j in range(NET // 2):
                        nc.tensor.matmul(
                            sps[ci][0:rows, 0:cw],
                            lhsT=qTt[j][:, 0:2, ib * P:ib * P + rows],
                            rhs=kTm[j][:, 0:2, c0:c0 + cw],
                            start=(j == 0),
                            stop=(j == NET // 2 - 1),
                            perf_mode=mybir.MatmulPerfMode.DoubleRow,
                        )
                else:
                    for et in range(NET):
                        nc.tensor.matmul(
                            sps[ci][0:rows, 0:cw],
                            lhsT=qTt[et][:, ib * P:ib * P + rows],
                            rhs=kTm[et][:, c0:c0 + cw],
                            start=(et == 0),
                            stop=(et == NET - 1),
                        )
                nc.scalar.activation(
                    out=exs[ci][0:rows, :], in_=sps[ci][0:rows, 0:cw],
                    func=AF.Exp, scale=SCALE, accum_out=rss[ci][0:rows, :],
                )
            rs = rss[0]
            for ci in range(1, nch):
                rst = stpool.tile([P, 1], F32, tag="rowsumt", name=f"frt{ci}")
                nc.vector.tensor_tensor(
                    out=rst[0:rows, :], in0=rs[0:rows, :],
                    in1=rss[ci][0:rows, :],
                    op=mybir.AluOpType.add)
                rs = rst
            rc = s_stats(rs, pad_sb, rows)
            for ci, (c0, cw) in enumerate(kchunks):
                poh = opool.tile([P, cw], BF16, tag=f"fpo{ci}", name=f"fpo{ci}")
                nc.vector.tensor_scalar(
                    out=poh[0:rows, :], in0=exs[ci][0:rows, :],
                    scalar1=rc[0:rows, :], scalar2=None,
                    op0=mybir.AluOpType.mult,
                )
                if cw > 256:
                    # split the store across two queues so the final
                    # transfers drain 2x faster.  scalar only on the very
                    # last block -- earlier its queue still owes exps.
                    h = cw // 2
                    eng2 = nc.scalar if last else nc.gpsimd
                    nc.sync.dma_start(
                        out=out[b, ib * P:ib * P + rows, c0:c0 + h],
                        in_=poh[0:rows, 0:h])
                    eng2.dma_start(
                        out=out[b, ib * P:ib * P + rows, c0 + h:c0 + cw],
                        in_=poh[0:rows, h:cw])
                else:
                    nc.gpsimd.dma_start(
                        out=out[b, ib * P:ib * P + rows, c0:c0 + cw],
                        in_=poh[0:rows, :],
                    )

        def s_phase(b, qTt, kTm, pad_sb):
            for ib in range(NQB):
                if b == BL - 1 and ib >= NQB - 2:
                    # last two blocks: per-chunk psum + split exp, so the
                    # Scalar queue drains before the final serial epilogue
                    s_block_final(b, ib, qTt, kTm, pad_sb,
                                  last=(ib == NQB - 1))
                else:
                    s_block(b, ib, qTt, kTm, pad_sb)

        cur = load_inputs(0)
        for b in range(BL):
            xk, xq, pad_sb, mask_sb = cur
            if use_mask:
                ktag = "kraw"
            else:
                ktag = "kTm"
            if s_fp8:
                kraw = [actpool.tile([P, 2, NKP], FP8, tag=f"{ktag}{j}",
                                     name=f"{ktag}{j}_{b}")
                        for j in range(NET // 2)]
            else:
                kraw = [actpool.tile([P, NKP], BF16, tag=f"{ktag}{et}",
                                     name=f"{ktag}{et}_{b}")
                        for et in range(NET)]
            if b == 0:
                proj_coldstart(xk, wk_sb, bk_sb, kraw, kchunks, pfx="coldk")
            else:
                proj(xk, wk_sb, bk_sb, kraw, kchunks)
            kTm = mask_add(kraw, mask_sb, b) if use_mask else kraw
            if s_fp8:
                qTt = [actpool.tile([P, 2, NQP], FP8, tag=f"qT{j}",
                                    name=f"qT{j}_{b}")
                       for j in range(NET // 2)]
            else:
                qTt = [actpool.tile([P, NQP], BF16, tag=f"qT{et}",
                                    name=f"qT{et}_{b}")
                       for et in range(NET)]
            if b == 0:
                proj_coldstart(xq, wq_sb, bq_sb, qTt, qchunks, pfx="coldq")
            else:
                proj(xq, wq_sb, bq_sb, qTt, qchunks)
            if b + 1 < BL:
                cur = load_inputs(b + 1)
            s_phase(b, qTt, kTm, pad_sb)


def _build(NQP, NKP, use_mask):
    nc = bacc.Bacc(
        "TRN2",
        target_bir_lowering=False,
        debug=False,
        enable_asserts=False,
        num_devices=NCORES,
    )
    qT = nc.dram_tensor("qT", [BL, D, NQP], BF16, kind="ExternalInput").ap()
    kT = nc.dram_tensor("kT", [BL, D, NKP], BF16, kind="ExternalInput").ap()
    Wq = nc.dram_tensor("Wq", [D, D], BF16, kind="ExternalInput").ap()
    Wk = nc.dram_tensor("Wk", [D, D], BF16, kind="ExternalInput").ap()
    bq = nc.dram_tensor("bq", [P, NET], F32, kind="ExternalInput").ap()
    bk = nc.dram_tensor("bk", [P, NET], F32, kind="ExternalInput").ap()
    padc = nc.dram_tensor("padc", [BL, P, 1], F32, kind="ExternalInput").ap()
    maskc = None
    if use_mask:
        maskc = nc.dram_tensor(
            "maskc", [BL, P, NKP], BF16, kind="ExternalInput").ap()
    out = nc.dram_tensor("out", [BL, NQP, NKP], BF16, kind="ExternalOutput").ap()

    with tile.TileContext(nc) as tc:
        _body(tc, qT, kT, Wq, Wk, bq, bk, padc, maskc, out, NQP, NKP)
    nc.compile()
    return nc


def _get_nc(NQP, NKP, use_mask):
    key = (NQP, NKP, use_mask)
    if key not in _CACHE:
        _CACHE[key] = _build(*key)
    return _CACHE[key]


def _pad64(n):
    # 64-col granularity: tail matmuls are free-dim-priced (no LDW floor),
    # so finer padding directly cuts PE cycles.  S-blocks still span 128
    # rows; a trailing 64-row block costs the same per column.
    return max(64, ((n + 63) // 64) * 64)


def _prep(query, key, query_mask, key_mask, Wq, bq, Wk, bk):
    bf = ml_dtypes.bfloat16
    query = np.asarray(query, dtype=np.float32)
    key = np.asarray(key, dtype=np.float32)
    qmask = np.asarray(query_mask) != 0
    kmask = np.asarray(key_mask) != 0
    qidx = [np.nonzero(qmask[g])[0] for g in range(B)]
    kidx = [np.nonzero(kmask[g])[0] for g in range(B)]
    NQP = _pad64(max(len(i) for i in qidx))
    NKP = _pad64(max(len(i) for i in kidx))
    use_mask = bool(np.any(np.asarray(bk, dtype=np.float32) != 0.0))

    Wq_b = np.asarray(Wq, dtype=np.float32).astype(bf)
    Wk_b = np.asarray(Wk, dtype=np.float32).astype(bf)
    # bias for feature e lives at partition e%128, column e//128
    bq_t = np.asarray(bq, dtype=np.float32).reshape(NET, P).T.copy()
    bk_t = np.asarray(bk, dtype=np.float32).reshape(NET, P).T.copy()

    in_maps = []
    for c in range(NCORES):
        qTc = np.zeros((BL, D, NQP), dtype=bf)
        kTc = np.zeros((BL, D, NKP), dtype=bf)
        padc = np.zeros((BL, P, 1), dtype=np.float32)
        imap = {"qT": qTc, "kT": kTc, "Wq": Wq_b, "Wk": Wk_b,
                "bq": bq_t, "bk": bk_t, "padc": padc}
        if use_mask:
            mk = np.zeros((BL, P, NKP), dtype=bf)
            imap["maskc"] = mk
        for b in range(BL):
            g = c * BL + b
            qi, ki = qidx[g], kidx[g]
            qTc[b, :, :len(qi)] = query[g][qi].T.astype(bf)
            kTc[b, :, :len(ki)] = key[g][ki].T.astype(bf)
            if use_mask:
                imap["maskc"][b, :, len(ki):] = bf(MASKC)
            else:
                padc[b, :, 0] = float(NKP - len(ki))
        in_maps.append(imap)
    return in_maps, qidx, kidx, NQP, NKP, use_mask


def run(query, key, query_mask, key_mask, Wq, bq, Wk, bk, **kwargs):
    """Run on hardware; returns (output, BassKernelResults)."""
    in_maps, qidx, kidx, NQP, NKP, use_mask = _prep(
        query, key, query_mask, key_mask, Wq, bq, Wk, bk)
    nc = _get_nc(NQP, NKP, use_mask)
    res = run_bass_kernel_spmd(nc, in_maps, core_ids=list(range(NCORES)),
                               **kwargs)
    full = np.zeros((B, LQ, LK), dtype=np.float32)
    for c in range(NCORES):
        oc = res.results[c]["out"]
        for b in range(BL):
            g = c * BL + b
            qi, ki = qidx[g], kidx[g]
            full[g][np.ix_(qi, ki)] = oc[b][:len(qi), :len(ki)].astype(np.float32)
    return full, res


def kernel(query, key, query_mask, key_mask, Wq, bq, Wk, bk):
    full, _ = run(query, key, query_mask, key_mask, Wq, bq, Wk, bk)
    return full


# revision 19
# speedup vs baseline: 1.0493x; 1.0000x over previous
"""Masked attention-weight kernel (dense_transformer) for 8 TRN2 NeuronCores.

Computes, for inputs query/key [32,1024,512] f32, masks [32,1024] i32:
    q = relu(query @ Wq + bq); k = relu(key @ Wk + bk)
    w = softmax((q @ k^T)/sqrt(512) + key_mask_additive) * query_mask
Output: [32, 1024, 1024] f32.

Strategy: data-parallel over batch (4 batches/core, no collectives) PLUS
host-side mask compaction.  Masked key columns have weight exactly 0 in the
reference (exp(-1e9) underflows) and masked query rows are zeroed, so the
host gathers only the valid ~512 query rows / key columns per batch, pads
them to a fixed NQP/NKP (multiple of 64, 576 for this data), and the device
runs dense attention on the compacted [NQP, NKP] problem -- ~2.4x fewer
matmul cycles than the full [1024,1024].  The host scatters the compact
bf16 output back into a zero-filled full-size f32 array.

Padded key columns are all-zero inputs, so (with zero bias -- true for this
problem) their projected features are 0, their logits are 0, and each
contributes exp(0)=1 to the softmax row-sum; the device subtracts the
host-provided pad count from the row-sum before taking the reciprocal.
If the key bias were nonzero the host instead ships an additive -1e4
column mask applied to the projected k (use_mask variant).

Per-core pipeline, per batch (all matmuls bf16 with f32 PSUM):
  1. kTm[e,j] = relu(Wk.T @ keyT + bk): PE matmuls in (512,64) psum-bank
     chunks -> relu+bias epilogue alternating ACT/DVE.
  2. qT[e,i] likewise.  Batch 0 runs both projections dt-major across 8
     open psum chains so the PE consumes (w_dt, x_dt) DMA pairs in arrival
     order at cold start.
  3. Per 128-row block: S = qT.T @ kTm (PE), ACT exp with fused row-sum,
     DVE pad-correction + reciprocal, DVE scale, DMA out (stores alternate
     between the gpsimd and sync queues).
"""

import sys

sys.path.insert(0, "/opt/trn_rl_repo")

import numpy as np
import ml_dtypes
from contextlib import ExitStack

import concourse.tile as tile
from concourse import bacc, mybir
from concourse.bass_utils import run_bass_kernel_spmd

P = 128
B, LQ, LK, D = 32, 1024, 1024, 512
NCORES = 8
BL = B // NCORES          # batches per core
NDT = D // P              # contraction tiles for projections
NET = D // P              # output-feature tiles (= S contraction tiles)
SCALE = float(1.0 / np.sqrt(D))
MASKC = -1.0e4

F32 = mybir.dt.float32
BF16 = mybir.dt.bfloat16
FP8 = mybir.dt.float8e4
AF = mybir.ActivationFunctionType

_CACHE = {}


def _chunks(width):
    """Split a free width into psum-bank-aligned chunks (<=512 each)."""
    out, c0 = [], 0
    while c0 < width:
        cw = min(512, width - c0)
        out.append((c0, cw))
        c0 += cw
    return out


def _body(tc, qT, kT, Wq, Wk, bq, bk, padc, maskc, out, NQP, NKP):
    nc = tc.nc
    # fp8 DoubleRow S-matmul measured L2 err 1.9e-2 vs the 2e-2 gate --
    # only ~3us faster than bf16 (S phase is ACT-bound), so keep bf16.
    s_fp8 = False
    NQB = (NQP + P - 1) // P  # S blocks per batch (last may be 64 rows)
    rows_of = lambda ib: min(P, NQP - ib * P)
    SPAD = ((NKP + 511) // 512) * 512   # psum tile width (bank aligned)
    kchunks = _chunks(NKP)
    qchunks = _chunks(NQP)
    use_mask = maskc is not None
    with ExitStack() as ctx:
        consts = ctx.enter_context(tc.tile_pool(name="consts", bufs=1))
        wpool = ctx.enter_context(tc.tile_pool(name="w", bufs=1))
        inpool = ctx.enter_context(tc.tile_pool(name="inp", bufs=2))
        actpool = ctx.enter_context(tc.tile_pool(name="act", bufs=2))
        mpool = ctx.enter_context(tc.tile_pool(name="mask", bufs=2))
        epool = ctx.enter_context(tc.tile_pool(name="exp", bufs=3))
        opool = ctx.enter_context(tc.tile_pool(name="pout", bufs=3))
        stpool = ctx.enter_context(tc.tile_pool(name="stat", bufs=6))
        ppsum = ctx.enter_context(tc.tile_pool(name="ppsum", bufs=3, space="PSUM"))
        spsum = ctx.enter_context(tc.tile_pool(name="spsum", bufs=2, space="PSUM"))
        cpsum = ctx.enter_context(tc.tile_pool(name="cpsum", bufs=1, space="PSUM"))

        # Weights on the scalar DMA queue, inputs on sync, small tensors on
        # gpsimd -- three queues pull concurrently at cold start.
        wk_sb = [wpool.tile([P, D], BF16, tag=f"wk{dt_}", name=f"wk{dt_}")
                 for dt_ in range(NDT)]
        wq_sb = [wpool.tile([P, D], BF16, tag=f"wq{dt_}", name=f"wq{dt_}")
                 for dt_ in range(NDT)]
        for dt_ in range(NDT):
            nc.scalar.dma_start(
                out=wk_sb[dt_][:], in_=Wk[dt_ * P:(dt_ + 1) * P, :])
        for dt_ in range(NDT):
            nc.scalar.dma_start(
                out=wq_sb[dt_][:], in_=Wq[dt_ * P:(dt_ + 1) * P, :])

        bk_sb = consts.tile([P, NET], F32)
        nc.gpsimd.dma_start(out=bk_sb[:], in_=bk[:])
        bq_sb = consts.tile([P, NET], F32)
        nc.gpsimd.dma_start(out=bq_sb[:], in_=bq[:])

        # PE warmup: 8 dummy matmuls (~3.4us of cold PE busy, exactly one
        # HAM activity window) on scratch tiles while the first input DMAs
        # are in flight, so the clock-gate reaches K=8/8 just before real
        # matmuls start. Results are never read.
        warm_in = consts.tile([P, 512], BF16, name="warm_in")
        nc.vector.memset(warm_in[:], 0.0)
        # 5 x 512-free at cold clock ends ~10.9us, just before the first
        # input tiles land (~11.1us) -- max ramp with zero real-MM delay
        warm_ps = ppsum.tile([P, 512], F32, tag="proj", name="warm_ps")
        for _ in range(5):
            nc.tensor.matmul(
                warm_ps[:], lhsT=warm_in[:, 0:P], rhs=warm_in[:],
                start=True, stop=True,
            )

        def load_inputs(b):
            xk, xq = [], []
            for dt_ in range(NDT):
                t = inpool.tile([P, NKP], BF16, tag=f"xk{dt_}")
                if b == 0:
                    # split per chunk: the cold-start proj chain is gated by
                    # ARRIVAL of each chunk, not by descriptor issue count
                    for (c0, cw) in kchunks:
                        nc.sync.dma_start(
                            out=t[:, c0:c0 + cw],
                            in_=kT[b, dt_ * P:(dt_ + 1) * P, c0:c0 + cw])
                else:
                    nc.sync.dma_start(
                        out=t[:], in_=kT[b, dt_ * P:(dt_ + 1) * P, :])
                xk.append(t)
            for dt_ in range(NDT):
                t = inpool.tile([P, NQP], BF16, tag=f"xq{dt_}")
                if b == 0:
                    # gpsimd queue: nearly idle at cold start, so xq doesn't
                    # serialize behind the xk loads on sync
                    for (c0, cw) in qchunks:
                        nc.gpsimd.dma_start(
                            out=t[:, c0:c0 + cw],
                            in_=qT[b, dt_ * P:(dt_ + 1) * P, c0:c0 + cw])
                else:
                    nc.sync.dma_start(
                        out=t[:], in_=qT[b, dt_ * P:(dt_ + 1) * P, :])
                xq.append(t)
            pad_sb = mpool.tile([P, 1], F32, tag="padc")
            nc.gpsimd.dma_start(out=pad_sb[:], in_=padc[b])
            mask_sb = None
            if use_mask:
                mask_sb = mpool.tile([P, NKP], BF16, tag="maskc")
                nc.gpsimd.dma_start(out=mask_sb[:], in_=maskc[b])
            return xk, xq, pad_sb, mask_sb

        def _eslice(out_tiles, et, c0, cw):
            # fp8 mode packs et pairs into [P, 2, N] DoubleRow operand tiles
            if s_fp8:
                return out_tiles[et // 2][:, et % 2, c0:c0 + cw]
            return out_tiles[et][:, c0:c0 + cw]

        def relu_epilogue(ps, bias_sb, out_tiles, et, c0, cw, on_dve):
            if on_dve:
                # (psum + bias) max 0 -- exact relu+bias as one DVE op
                nc.vector.tensor_scalar(
                    out=_eslice(out_tiles, et, c0, cw),
                    in0=ps,
                    scalar1=bias_sb[:, et:et + 1],
                    scalar2=0.0,
                    op0=mybir.AluOpType.add,
                    op1=mybir.AluOpType.max,
                )
            else:
                nc.scalar.activation(
                    out=_eslice(out_tiles, et, c0, cw),
                    in_=ps,
                    func=AF.Relu,
                    bias=bias_sb[:, et:et + 1],
                    scale=1.0,
                )

        def proj(xin, w_sb, bias_sb, out_tiles, chunks):
            # out_tiles[et] = relu(W[:, et].T @ x + b)
            n = 0
            for et in range(NET):
                for (c0, cw) in chunks:
                    ps = ppsum.tile([P, 512], F32, tag="proj")
                    for dt_ in range(NDT):
                        nc.tensor.matmul(
                            ps[:, 0:cw],
                            lhsT=w_sb[dt_][:, et * P:(et + 1) * P],
                            rhs=xin[dt_][:, c0:c0 + cw],
                            start=(dt_ == 0),
                            stop=(dt_ == NDT - 1),
                        )
                    relu_epilogue(ps[:, 0:cw], bias_sb, out_tiles, et, c0, cw,
                                  on_dve=(n % 2 == 1))
                    n += 1

        def proj_coldstart(xin, w_sb, bias_sb, out_tiles, chunks, pfx):
            # Batch-0 projections: dt-major order so the PE consumes
            # (w_dt, x_dt) DMA pairs in arrival order; all NET*len(chunks)
            # accumulation chains are open at once, borrowing the (still
            # idle) S-phase psum pool.  Chain -> single-bank psum region:
            #   chunk0 (512 wide) x4 et -> spsum tiles 0,1 (two banks each)
            #   chunk1 (<=128)    x4 et -> ppsum x3 + cpsum
            sp0 = spsum.tile([P, SPAD], F32, tag="S", name=f"{pfx}c0a")
            sp1 = spsum.tile([P, SPAD], F32, tag="S", name=f"{pfx}c0b")
            big = [sp0[:, 0:512], sp0[:, 512:1024],
                   sp1[:, 0:512], sp1[:, 512:1024]]
            regions = {}
            for et in range(NET):
                regions[(et, 0)] = big[et]
            if len(chunks) > 1:
                cw1 = chunks[1][1]
                pps = [ppsum.tile([P, 512], F32, tag="proj",
                                  name=f"{pfx}c1{i}") for i in range(3)]
                pps.append(cpsum.tile([P, 512], F32, tag="cold",
                                      name=f"{pfx}c1x"))
                for et in range(NET):
                    regions[(et, 1)] = pps[et][:, 0:cw1]
            for dt_ in range(NDT):
                for et in range(NET):
                    for ci, (c0, cw) in enumerate(chunks):
                        nc.tensor.matmul(
                            regions[(et, ci)],
                            lhsT=w_sb[dt_][:, et * P:(et + 1) * P],
                            rhs=xin[dt_][:, c0:c0 + cw],
                            start=(dt_ == 0),
                            stop=(dt_ == NDT - 1),
                        )
            # chunk-major epilogues: S block 0 needs cols 0:128 of every et
            # tile, which chunk 0 covers -- drain those four chains first
            n = 0
            for ci, (c0, cw) in enumerate(chunks):
                for et in range(NET):
                    relu_epilogue(regions[(et, ci)], bias_sb, out_tiles,
                                  et, c0, cw, on_dve=(n % 2 == 1))
                    n += 1

        def mask_add(kraw, mask_sb, b):
            kTm = [actpool.tile([P, NKP], BF16, tag=f"kTm{et}",
                                name=f"kTm{et}_{b}")
                   for et in range(NET)]
            for et in range(NET):
                # split across gpsimd and vector so neither gates the S phase
                eng = nc.gpsimd if et % 2 == 0 else nc.vector
                eng.tensor_add(kTm[et][:], kraw[et][:], mask_sb[:])
            return kTm

        def s_stats(rs, pad_sb, rows=P):
            # row-sum -> subtract pad-column contribution -> reciprocal
            # (all on DVE: a cross-engine sub->recip chain measurably
            # stalls DVE head-of-line behind GpSimd's store issues)
            rsv = stpool.tile([P, 1], F32, tag="rsv")
            nc.vector.tensor_tensor(
                out=rsv[0:rows, :], in0=rs[0:rows, :], in1=pad_sb[0:rows, :],
                op=mybir.AluOpType.subtract,
            )
            rc = stpool.tile([P, 1], F32, tag="recip")
            nc.vector.reciprocal(out=rc[0:rows, :], in_=rsv[0:rows, :])
            return rc

        def s_block(b, ib, qTt, kTm, pad_sb):
            rows = rows_of(ib)
            sp = spsum.tile([P, SPAD], F32, tag="S")
            for (c0, cw) in kchunks:
                if s_fp8:
                    for j in range(NET // 2):
                        nc.tensor.matmul(
                            sp[0:rows, c0:c0 + cw],
                            lhsT=qTt[j][:, 0:2, ib * P:ib * P + rows],
                            rhs=kTm[j][:, 0:2, c0:c0 + cw],
                            start=(j == 0),
                            stop=(j == NET // 2 - 1),
                            perf_mode=mybir.MatmulPerfMode.DoubleRow,
                        )
                else:
                    for et in range(NET):
                        nc.tensor.matmul(
                            sp[0:rows, c0:c0 + cw],
                            lhsT=qTt[et][:, ib * P:ib * P + rows],
                            rhs=kTm[et][:, c0:c0 + cw],
                            start=(et == 0),
                            stop=(et == NET - 1),
                        )
            ex = epool.tile([P, NKP], BF16, tag="exp")
            rs = stpool.tile([P, 1], F32, tag="rowsum")
            nc.scalar.activation(
                out=ex[0:rows, :], in_=sp[0:rows, 0:NKP], func=AF.Exp,
                scale=SCALE, accum_out=rs[0:rows, :],
            )
            rc = s_stats(rs, pad_sb, rows)
            po = opool.tile([P, NKP], BF16, tag="po")
            nc.vector.tensor_scalar(
                out=po[0:rows, :], in0=ex[0:rows, :],
                scalar1=rc[0:rows, :], scalar2=None,
                op0=mybir.AluOpType.mult,
            )
            # alternate store queues so the output backlog drains 2x faster
            # (sync, not scalar: scalar's ACT must not stall behind DMA issue)
            eng = nc.gpsimd if ib % 2 == 0 else nc.sync
            eng.dma_start(out=out[b, ib * P:ib * P + rows, :],
                          in_=po[0:rows, :])

        def s_block_final(b, ib, qTt, kTm, pad_sb, last=True):
            # Last block of the kernel: chunk-major matmuls into separate
            # 1-bank psums + a fully split epilogue so the first chunk's
            # exp/mul/store overlap the second chunk's matmuls and exp --
            # shortening the serial tail after the last MM.
            rows = rows_of(ib)
            nch = len(kchunks)
            sps, rss, exs = [], [], []
            for ci, (c0, cw) in enumerate(kchunks):
                sps.append(ppsum.tile([P, 512], F32, tag="proj",
                                      name=f"fsp{ci}"))
                rss.append(stpool.tile([P, 1], F32, tag=f"rowsum{ci}",
                                       name=f"frs{ci}"))
                exs.append(epool.tile([P, cw], BF16, tag=f"fex{ci}",
                                      name=f"fex{ci}"))
            for ci, (c0, cw) in enumerate(kchunks):
                if s_fp8:
                    for j in range(NET // 2):
                        nc.tensor.matmul(
                            sps[ci][0:rows, 0:cw],
                            lhsT=qTt[j][:, 0:2, ib * P:ib * P + rows],
                            rhs=kTm[j][:, 0:2, c0:c0 + cw],
                            start=(j == 0),
                            stop=(j == NET // 2 - 1),
                            perf_mode=mybir.MatmulPerfMode.DoubleRow,
                        )
                else:
                    for et in range(NET):
                        nc.tensor.matmul(
                            sps[ci][0:rows, 0:cw],
                            lhsT=qTt[et][:, ib * P:ib * P + rows],
                            rhs=kTm[et][:, c0:c0 + cw],
                            start=(et == 0),
                            stop=(et == NET - 1),
                        )
                nc.scalar.activation(
                    out=exs[ci][0:rows, :], in_=sps[ci][0:rows, 0:cw],
                    func=AF.Exp, scale=SCALE, accum_out=rss[ci][0:rows, :],
                )
            rs = rss[0]
            for ci in range(1, nch):
                rst = stpool.tile([P, 1], F32, tag="rowsumt", name=f"frt{ci}")
                nc.vector.tensor_tensor(
                    out=rst[0:rows, :], in0=rs[0:rows, :],
                    in1=rss[ci][0:rows, :],
                    op=mybir.AluOpType.add)
                rs = rst
            rc = s_stats(rs, pad_sb, rows)
            for ci, (c0, cw) in enumerate(kchunks):
                poh = opool.tile([P, cw], BF16, tag=f"fpo{ci}", name=f"fpo{ci}")
                nc.vector.tensor_scalar(
                    out=poh[0:rows, :], in0=exs[ci][0:rows, :],
                    scalar1=rc[0:rows, :], scalar2=None,
                    op0=mybir.AluOpType.mult,
                )
                if cw > 256:
                    # split the store across two queues so the final
                    # transfers drain 2x faster.  scalar only on the very
                    # last block -- earlier its queue still owes exps.
                    h = cw // 2
                    eng2 = nc.scalar if last else nc.gpsimd
                    nc.sync.dma_start(
                        out=out[b, ib * P:ib * P + rows, c0:c0 + h],
                        in_=poh[0:rows, 0:h])
                    eng2.dma_start(
                        out=out[b, ib * P:ib * P + rows, c0 + h:c0 + cw],
                        in_=poh[0:rows, h:cw])
                else:
                    nc.gpsimd.dma_start(
                        out=out[b, ib * P:ib * P + rows, c0:c0 + cw],
                        in_=poh[0:rows, :],
                    )

        def s_phase(b, qTt, kTm, pad_sb):
            for ib in range(NQB):
                if b == BL - 1 and ib >= NQB - 2:
                    # last two blocks: per-chunk psum + split exp, so the
                    # Scalar queue drains before the final serial epilogue
                    s_block_final(b, ib, qTt, kTm, pad_sb,
                                  last=(ib == NQB - 1))
                else:
                    s_block(b, ib, qTt, kTm, pad_sb)

        cur = load_inputs(0)
        for b in range(BL):
            xk, xq, pad_sb, mask_sb = cur
            if use_mask:
                ktag = "kraw"
            else:
                ktag = "kTm"
            if s_fp8:
                kraw = [actpool.tile([P, 2, NKP], FP8, tag=f"{ktag}{j}",
                                     name=f"{ktag}{j}_{b}")
                        for j in range(NET // 2)]
            else:
                kraw = [actpool.tile([P, NKP], BF16, tag=f"{ktag}{et}",
                                     name=f"{ktag}{et}_{b}")
                        for et in range(NET)]
            if b == 0:
                proj_coldstart(xk, wk_sb, bk_sb, kraw, kchunks, pfx="coldk")
            else:
                proj(xk, wk_sb, bk_sb, kraw, kchunks)
            kTm = mask_add(kraw, mask_sb, b) if use_mask else kraw
            if s_fp8:
                qTt = [actpool.tile([P, 2, NQP], FP8, tag=f"qT{j}",
                                    name=f"qT{j}_{b}")
                       for j in range(NET // 2)]
            else:
                qTt = [actpool.tile([P, NQP], BF16, tag=f"qT{et}",
                                    name=f"qT{et}_{b}")
                       for et in range(NET)]
            if b == 0:
                proj_coldstart(xq, wq_sb, bq_sb, qTt, qchunks, pfx="coldq")
            else:
                proj(xq, wq_sb, bq_sb, qTt, qchunks)
            if b + 1 < BL:
                cur = load_inputs(b + 1)
            s_phase(b, qTt, kTm, pad_sb)


def _build(NQP, NKP, use_mask):
    nc = bacc.Bacc(
        "TRN2",
        target_bir_lowering=False,
        debug=False,
        enable_asserts=False,
        num_devices=NCORES,
    )
    qT = nc.dram_tensor("qT", [BL, D, NQP], BF16, kind="ExternalInput").ap()
    kT = nc.dram_tensor("kT", [BL, D, NKP], BF16, kind="ExternalInput").ap()
    Wq = nc.dram_tensor("Wq", [D, D], BF16, kind="ExternalInput").ap()
    Wk = nc.dram_tensor("Wk", [D, D], BF16, kind="ExternalInput").ap()
    bq = nc.dram_tensor("bq", [P, NET], F32, kind="ExternalInput").ap()
    bk = nc.dram_tensor("bk", [P, NET], F32, kind="ExternalInput").ap()
    padc = nc.dram_tensor("padc", [BL, P, 1], F32, kind="ExternalInput").ap()
    maskc = None
    if use_mask:
        maskc = nc.dram_tensor(
            "maskc", [BL, P, NKP], BF16, kind="ExternalInput").ap()
    out = nc.dram_tensor("out", [BL, NQP, NKP], BF16, kind="ExternalOutput").ap()

    with tile.TileContext(nc) as tc:
        _body(tc, qT, kT, Wq, Wk, bq, bk, padc, maskc, out, NQP, NKP)
    nc.compile()
    return nc


def _get_nc(NQP, NKP, use_mask):
    key = (NQP, NKP, use_mask)
    if key not in _CACHE:
        _CACHE[key] = _build(*key)
    return _CACHE[key]


def _pad64(n):
    # 64-col granularity: tail matmuls are free-dim-priced (no LDW floor),
    # so finer padding directly cuts PE cycles.  S-blocks still span 128
    # rows; a trailing 64-row block costs the same per column.
    return max(64, ((n + 63) // 64) * 64)


def _prep(query, key, query_mask, key_mask, Wq, bq, Wk, bk):
    bf = ml_dtypes.bfloat16
    query = np.asarray(query, dtype=np.float32)
    key = np.asarray(key, dtype=np.float32)
    qmask = np.asarray(query_mask) != 0
    kmask = np.asarray(key_mask) != 0
    qidx = [np.nonzero(qmask[g])[0] for g in range(B)]
    kidx = [np.nonzero(kmask[g])[0] for g in range(B)]
    NQP = _pad64(max(len(i) for i in qidx))
    NKP = _pad64(max(len(i) for i in kidx))
    use_mask = bool(np.any(np.asarray(bk, dtype=np.float32) != 0.0))

    Wq_b = np.asarray(Wq, dtype=np.float32).astype(bf)
    Wk_b = np.asarray(Wk, dtype=np.float32).astype(bf)
    # bias for feature e lives at partition e%128, column e//128
    bq_t = np.asarray(bq, dtype=np.float32).reshape(NET, P).T.copy()
    bk_t = np.asarray(bk, dtype=np.float32).reshape(NET, P).T.copy()

    in_maps = []
    for c in range(NCORES):
        qTc = np.zeros((BL, D, NQP), dtype=bf)
        kTc = np.zeros((BL, D, NKP), dtype=bf)
        padc = np.zeros((BL, P, 1), dtype=np.float32)
        imap = {"qT": qTc, "kT": kTc, "Wq": Wq_b, "Wk": Wk_b,
                "bq": bq_t, "bk": bk_t, "padc": padc}
        if use_mask:
            mk = np.zeros((BL, P, NKP), dtype=bf)
            imap["maskc"] = mk
        for b in range(BL):
            g = c * BL + b
            qi, ki = qidx[g], kidx[g]
            qTc[b, :, :len(qi)] = query[g][qi].T.astype(bf)
            kTc[b, :, :len(ki)] = key[g][ki].T.astype(bf)
            if use_mask:
                imap["maskc"][b, :, len(ki):] = bf(MASKC)
            else:
                padc[b, :, 0] = float(NKP - len(ki))
        in_maps.append(imap)
    return in_maps, qidx, kidx, NQP, NKP, use_mask


def run(query, key, query_mask, key_mask, Wq, bq, Wk, bk, **kwargs):
    """Run on hardware; returns (output, BassKernelResults)."""
    in_maps, qidx, kidx, NQP, NKP, use_mask = _prep(
        query, key, query_mask, key_mask, Wq, bq, Wk, bk)
    nc = _get_nc(NQP, NKP, use_mask)
    res = run_bass_kernel_spmd(nc, in_maps, core_ids=list(range(NCORES)),
                               **kwargs)
    full = np.zeros((B, LQ, LK), dtype=np.float32)
    for c in range(NCORES):
        oc = res.results[c]["out"]
        for b in range(BL):
            g = c * BL + b
            qi, ki = qidx[g], kidx[g]
            full[g][np.ix_(qi, ki)] = oc[b][:len(qi), :len(ki)].astype(np.float32)
    return full, res


def kernel(query, key, query_mask, key_mask, Wq, bq, Wk, bk):
    full, _ = run(query, key, query_mask, key_mask, Wq, bq, Wk, bk)
    return full
